# revision 1
# baseline (speedup 1.0000x reference)
"""GCN (2-layer, edge-weighted, log_softmax) on 8 Trainium2 NeuronCores.

Strategy (dst-sharded edges, matmul-based segment-sum):
  - Nodes sharded 12544/core (table rows = node ids, 352 junk tail rows).
  - Layer 1: h = x @ W1 computed data-parallel on node shards -> AllGather
    into a replicated, 256B-strided feature table in HBM.
  - Per-edge gather of 128B rows via the InstDMAGatherAnt SWDGE ucode
    (int16 idx => 4 table chunks of 25088 rows; edges grouped by src chunk).
  - Edges packed into 8-edge same-destination slots; DVE does x-weight and
    an 8->1 tree reduction; a per-column one-hot (is_equal vs iota) matmul
    segment-sums slot partials into PSUM windows of 128 destinations,
    accumulated into an SBUF aggregate laid out [d%128, (d//128)*32+f].
  - Layer 2 aggregates relu(agg1 + b1) with the identical edge structure,
    then applies W2 (+b2 via a ones-row matmul) per 128-node window,
    followed by an on-chip log_softmax.
Host side only packs indices/weights (numpy) and concatenates shards.
"""

import os
import sys

for _p in ("/opt/trn_rl_repo", "/root/.axon_site/_ro/trn_rl_repo"):
    if os.path.isdir(_p) and _p not in sys.path:
        sys.path.insert(0, _p)

import numpy as np

import concourse.ap_utils as ap_utils
import concourse.bass as bass
import concourse.mybir as mybir
from concourse import bacc, tile
from concourse.bass_utils import run_bass_kernel_spmd

CORES = 8
F_IN = 128
F_HID = 32
F_OUT = 40
KSLOT = 8  # edges per slot (same destination)
GK_INSTR = 64  # gather k-columns per instruction (8192 tokens)


class Geo:
    """Problem geometry. Full size by default; shrinkable for simulation."""

    def __init__(self, n_nodes=100000, nsh=12544, chunk=25088, groups=4):
        self.n_nodes = n_nodes
        self.nsh = nsh  # nodes per core shard (mult of 128)
        self.ntab = nsh * CORES  # table rows
        self.chunk = chunk  # gather table chunk rows (<= 32768)
        self.groups = groups
        assert chunk * groups == self.ntab
        assert nsh % 128 == 0
        self.nwin = nsh // 128  # 128-destination windows per core


FULL = Geo()


def _wrap16(flat, T):
    """token i -> [i%16, i//16], replicated to 128 partitions."""
    a = flat.reshape(T // 16, 16).T
    return np.tile(a, (8, 1)).copy()


def pack(edge_index, edge_weight, geo: Geo):
    """Group edges by (core, src-chunk, dst) into 8-edge slots; build the
    shared column->window template and all per-core device arrays."""
    src = np.asarray(edge_index[0], dtype=np.int64)
    dst = np.asarray(edge_index[1], dtype=np.int64)
    w = np.asarray(edge_weight, dtype=np.float32)
    nsh, nwin, G = geo.nsh, geo.nwin, geo.groups

    core = dst // nsh
    pc = []
    cnt = np.zeros((CORES, G, nwin), np.int64)
    for c in range(CORES):
        m = core == c
        s_c = src[m]
        dl = dst[m] - c * nsh
        wc = w[m]
        g = s_c // geo.chunk
        key = g * (2 * nsh) + dl
        order = np.argsort(key, kind="stable")
        sk = (s_c - g * geo.chunk)[order]
        dlk = dl[order]
        wk = wc[order]
        kk = key[order]
        new = np.r_[True, kk[1:] != kk[:-1]]
        run_first = np.flatnonzero(new)
        run_len = np.diff(np.r_[run_first, len(kk)])
        run_id = np.cumsum(new) - 1
        rank = np.arange(len(kk)) - run_first[run_id]
        nsl = (run_len + KSLOT - 1) // KSLOT
        g_run = (g[order])[run_first]
        dl_run = dlk[run_first]
        v_run = dl_run // 128
        np.add.at(cnt[c], (g_run, v_run), nsl)
        pc.append((sk, dlk, wk, rank, run_id, nsl, g_run, dl_run, v_run))

    # shared template
    cap = cnt.max(axis=0)  # [G, nwin]
    cap = ((cap + 31) // 32) * 32
    gslots = cap.sum(axis=1)
    gpad = (-gslots) % 128
    off = np.zeros((G, nwin), np.int64)
    gbase = np.zeros(G + 1, np.int64)
    b = 0
    for g in range(G):
        gbase[g] = b
        for v in range(nwin):
            off[g, v] = b
            b += cap[g, v]
        b += gpad[g]
    gbase[G] = b
    S_T = int(b)
    COLS = S_T // 128
    T = S_T * KSLOT
    KC = T // 128  # k-columns total

    # per-column window lists (template)
    colmeta = []  # (A_col, [wins]) or None
    flat_off = []
    flat_gv = []
    for g in range(G):
        for v in range(nwin):
            if cap[g, v] > 0:
                flat_off.append(int(off[g, v]))
                flat_gv.append((g, v))
    flat_off = np.array(flat_off + [S_T], dtype=np.int64)
    for col in range(COLS):
        lo, hi = col * 128, col * 128 + 128
        i0 = int(np.searchsorted(flat_off, lo, side="right") - 1)
        wins = []
        gcol = None
        for i in range(max(i0, 0), len(flat_gv)):
            o = flat_off[i]
            if o >= hi:
                break
            o2 = flat_off[i + 1]
            if o2 <= lo:
                continue
            gg, vv = flat_gv[i]
            if gcol is None:
                gcol = gg
            if gg == gcol:
                wins.append(vv)
        # group-tail pad regions have no (g,v); wins may be empty
        colmeta.append((wins[0], wins, gcol) if wins else None)

    # per-group k-column ranges for gather instructions
    ginstr = []  # (group, kc0, gk)
    for g in range(G):
        kc_lo = int(gbase[g]) // 16
        kc_hi = int(gbase[g + 1]) // 16
        kc = kc_lo
        while kc < kc_hi:
            gk = min(GK_INSTR, kc_hi - kc)
            ginstr.append((g, kc, gk))
            kc += gk

    # per-core arrays
    inmaps = []
    for c in range(CORES):
        sk, dlk, wk, rank, run_id, nsl, g_run, dl_run, v_run = pc[c]
        n_runs = len(nsl)
        csum = np.cumsum(nsl)
        start_excl = np.r_[0, csum[:-1]]
        gv = g_run * nwin + v_run
        newgv = np.r_[True, gv[1:] != gv[:-1]]
        gv_first = np.flatnonzero(newgv)
        gv_id = np.cumsum(newgv) - 1
        base_in_gv = start_excl - start_excl[gv_first][gv_id]
        run_slot = off[g_run, v_run] + base_in_gv
        slot_e = run_slot[run_id] + rank // KSLOT
        j_e = rank % KSLOT
        cs_e = slot_e // 128
        p_e = slot_e % 128
        tok = (cs_e * KSLOT + j_e) * 128 + p_e

        idx_flat = np.zeros(T, np.int16)
        idx_flat[tok] = sk.astype(np.int16)
        w_flat = np.zeros(T, np.float32)
        w_flat[tok] = wk

        dl_slot = np.full(S_T, 100000.0, np.float32)
        reps = np.repeat(np.arange(n_runs), nsl)
        ar = np.arange(len(reps)) - np.repeat(start_excl, nsl)
        pos = np.repeat(run_slot, nsl) + ar
        dl_slot[pos] = np.repeat(dl_run, nsl).astype(np.float32)
        dcol = dl_slot.reshape(COLS, 128).T.copy()  # [128, COLS]
        for col in range(COLS):
            if colmeta[col] is not None:
                dcol[:, col] -= 128.0 * colmeta[col][0]

        inmaps.append(
            dict(
                gidx=_wrap16(idx_flat, T),
                wgrid=w_flat.reshape(KC, 128).T.copy(),
                dloc=dcol,
            )
        )

    meta = dict(S_T=S_T, COLS=COLS, T=T, KC=KC, colmeta=colmeta, ginstr=ginstr, geo=geo)
    return meta, inmaps


def emit_dma_gather(gp, out_ap, in_ap, idxs_ap, num_idxs, elem_size, elem_step):
    """bass.dma_gather minus the blanket 256B elem assert (verified on HW that
    the non-transpose ucode handles 128B rows)."""
    from concourse.bass import exact_div

    assert idxs_ap.dtype == mybir.dt.int16
    assert in_ap.dtype == out_ap.dtype
    assert in_ap.space == bass.MemorySpace.DRAM
    assert ap_utils.ap_is_contiguous(in_ap.ap[1:])
    assert ap_utils.ap_is_contiguous(out_ap.ap[1:])
    assert ap_utils.ap_is_contiguous(idxs_ap.ap[1:])
    assert in_ap.ap[-1][1] == out_ap.ap[-1][1] == elem_size
    assert out_ap.ap[0][1] * out_ap.ap[1][1] == num_idxs
    assert in_ap.ap[0][0] == elem_step
    stride_bytes_256 = exact_div(elem_step * mybir.dt.size(in_ap.dtype), 256)
    assert stride_bytes_256 < 256
    _in_ap = gp.lower_ap_dma(in_ap, for_custom_bir_dma=True)
    _idxs_ap = gp.lower_ap(idxs_ap)
    _out_ap = gp.lower_ap(out_ap)
    return gp.add_instruction(
        mybir.InstDMAGatherAnt(
            name=gp.bass.get_next_instruction_name(),
            ins=[*_in_ap, _idxs_ap, gp.lower_val_access(gp.to_reg(num_idxs))],
            outs=[_out_ap],
            transpose=False,
            num_idxs=num_idxs,
            elem_size=elem_size,
            stride_bytes_256=stride_bytes_256,
            gen_mode=0,
            single_packet=False,
            queue_num=0,
            sbuf_tokens_per_rank=0,
            sbuf_free_dim_per_rank=0,
            sbuf_free_dim_pad_per_rank=0,
            sbuf_byte_offset=0,
        )
    )


def _b(ap2, reps):
    """broadcast each element of a [P, K] AP over `reps` trailing copies."""
    return bass.AP(tensor=ap2.tensor, offset=ap2.offset, ap=[*ap2.ap, [0, reps]])


def _bcast_col(ap1, n):
    """[P, 1] AP -> [P, n] zero-stride broadcast (drops the unit free dim)."""
    return bass.AP(tensor=ap1.tensor, offset=ap1.offset, ap=[ap1.ap[0], [0, n]])


def build(meta):
    geo: Geo = meta["geo"]
    S_T, COLS, T, KC = meta["S_T"], meta["COLS"], meta["T"], meta["KC"]
    colmeta, ginstr = meta["colmeta"], meta["ginstr"]
    nsh, ntab, nwin, G = geo.nsh, geo.ntab, geo.nwin, geo.groups
    f32 = mybir.dt.float32
    AX = mybir.AxisListType.X
    AF = mybir.ActivationFunctionType

    nc = bacc.Bacc("TRN2", target_bir_lowering=False, debug=False, num_devices=CORES)

    xT = nc.dram_tensor("xT", [F_IN, nsh], f32, kind="ExternalInput")
    gidx = nc.dram_tensor("gidx", [128, T // 16], mybir.dt.int16, kind="ExternalInput")
    wgrid = nc.dram_tensor("wgrid", [128, KC], f32, kind="ExternalInput")
    dloc = nc.dram_tensor("dloc", [128, COLS], f32, kind="ExternalInput")
    iota512 = nc.dram_tensor("iota512", [128, 512], f32, kind="ExternalInput")
    ident = nc.dram_tensor("ident", [128, 128], f32, kind="ExternalInput")
    W1t = nc.dram_tensor("W1t", [F_IN, F_HID], f32, kind="ExternalInput")
    b1t = nc.dram_tensor("b1t", [128, F_HID], f32, kind="ExternalInput")
    W2t = nc.dram_tensor("W2t", [F_HID, F_OUT], f32, kind="ExternalInput")
    b2t = nc.dram_tensor("b2t", [1, F_OUT], f32, kind="ExternalInput")
    onest = nc.dram_tensor("onest", [1, 128], f32, kind="ExternalInput")
    out_t = nc.dram_tensor("out", [nsh, F_OUT], f32, kind="ExternalOutput")

    def shard_ap(tensor):
        # [128, v, f(32 of 64)] view of a [nsh, 64] shard: row = v*128 + p
        return bass.AP(tensor=tensor.tensor, offset=0, ap=[[64, 128], [128 * 64, nwin], [1, 32]])

    with tile.TileContext(nc) as tc:
        with (
            tc.tile_pool(name="const", bufs=1) as cpool,
            tc.tile_pool(name="dram", bufs=1, space="DRAM") as dram,
            tc.tile_pool(name="work", bufs=3) as wp,
            tc.tile_pool(name="scol", bufs=8) as sp,
            tc.tile_pool(name="agg", bufs=1) as apool,
        ):
            iota_t = cpool.tile([128, 512], f32)
            nc.sync.dma_start(out=iota_t[:], in_=iota512[:, :])
            dloc_t = cpool.tile([128, COLS], f32)
            nc.sync.dma_start(out=dloc_t[:], in_=dloc[:, :])
            W1s = cpool.tile([F_IN, F_HID], f32)
            nc.sync.dma_start(out=W1s[:], in_=W1t[:, :])
            b1s = cpool.tile([128, F_HID], f32)
            nc.sync.dma_start(out=b1s[:], in_=b1t[:, :])
            W2s = cpool.tile([F_HID, F_OUT], f32)
            nc.sync.dma_start(out=W2s[:], in_=W2t[:, :])
            b2s = cpool.tile([1, F_OUT], f32)
            nc.sync.dma_start(out=b2s[:], in_=b2t[:, :])
            ones_s = cpool.tile([1, 128], f32)
            nc.sync.dma_start(out=ones_s[:], in_=onest[:, :])
            id_s = cpool.tile([128, 128], f32)
            nc.sync.dma_start(out=id_s[:], in_=ident[:, :])

            shard1 = dram.tile([nsh, 64], f32)
            shard2 = dram.tile([nsh, 64], f32)
            table1 = dram.tile([ntab, 64], f32)
            table2 = dram.tile([ntab, 64], f32)

            # ---- h = x @ W1 on own shard -> shard1 ----
            with (
                tc.tile_pool(name="xt", bufs=1) as xp,
                tc.tile_pool(name="ph", bufs=2, space="PSUM") as ph,
            ):
                half = nsh // 2
                for hh in range(2):
                    xTs = xp.tile([F_IN, half], f32, tag="xts")
                    nc.sync.dma_start(out=xTs[:], in_=xT[:, hh * half : (hh + 1) * half])
                    for tt in range(half // 128):
                        t = hh * (half // 128) + tt
                        hp = ph.tile([128, F_HID], f32, tag="hps")
                        nc.tensor.matmul(
                            out=hp[:], lhsT=xTs[:, tt * 128 : (tt + 1) * 128], rhs=W1s[:],
                            start=True, stop=True,
                        )
                        hs = wp.tile([128, 64], f32, tag="hsb")
                        nc.vector.memset(hs[:], 0.0)
                        nc.vector.tensor_copy(out=hs[:, :F_HID], in_=hp[:])
                        dst = bass.AP(
                            tensor=shard1.tensor, offset=t * 128 * 64,
                            ap=[[64, 128], [1, 64]],
                        )
                        nc.sync.dma_start(out=dst, in_=hs[:])

            nc.gpsimd.collective_compute(
                "AllGather", mybir.AluOpType.bypass,
                ins=[shard1.opt()], outs=[table1.opt()],
                replica_groups=[list(range(CORES))],
            )

            def layer(table, agg_tile, init_b, ps2):
                if init_b is not None:
                    bsrc = bass.AP(
                        tensor=init_b.tensor, offset=init_b[:].offset,
                        ap=[init_b[:].ap[0], [0, nwin], [1, 32]],
                    )
                    nc.vector.tensor_copy(
                        out=agg_tile[:].rearrange("p (v f) -> p v f", f=32), in_=bsrc
                    )
                else:
                    nc.vector.memset(agg_tile[:], 0.0)
                for g, kc0, gk in ginstr:
                    gx = wp.tile([128, gk * 8], mybir.dt.int16, tag="gx")
                    nc.sync.dma_start(out=gx[:], in_=gidx[:, kc0 * 8 : (kc0 + gk) * 8])
                    ws = wp.tile([128, gk], f32, tag="ws")
                    nc.sync.dma_start(out=ws[:], in_=wgrid[:, kc0 : kc0 + gk])
                    msgs = wp.tile([128, gk * 32], f32, tag="msgs")
                    emit_dma_gather(
                        nc.gpsimd,
                        out_ap=msgs[:].rearrange("p (k f) -> p k f", f=32),
                        in_ap=bass.AP(
                            tensor=table.tensor, offset=g * geo.chunk * 64,
                            ap=[[64, geo.chunk], [1, 32]],
                        ),
                        idxs_ap=gx[:],
                        num_idxs=gk * 128,
                        elem_size=32,
                        elem_step=64,
                    )
                    nc.vector.tensor_tensor(
                        out=msgs[:].rearrange("p (k f) -> p k f", f=32),
                        in0=msgs[:].rearrange("p (k f) -> p k f", f=32),
                        in1=_b(ws[:], 32), op=mybir.AluOpType.mult,
                    )
                    # 8 -> 1 tree reduction over k within each slot
                    nb = gk  # 32-elem blocks
                    t1 = wp.tile([128, nb // 2 * 32], f32, tag="t1")
                    nc.vector.tensor_tensor(
                        out=t1[:].rearrange("p (k f) -> p k f", f=32),
                        in0=bass.AP(tensor=msgs.tensor, offset=msgs[:].offset,
                                               ap=[msgs[:].ap[0], [64, nb // 2], [1, 32]]),
                        in1=bass.AP(tensor=msgs.tensor, offset=msgs[:].offset + 32,
                                    ap=[msgs[:].ap[0], [64, nb // 2], [1, 32]]),
                        op=mybir.AluOpType.add,
                    )
                    t2 = wp.tile([128, nb // 4 * 32], f32, tag="t2")
                    nc.vector.tensor_tensor(
                        out=t2[:].rearrange("p (k f) -> p k f", f=32),
                        in0=bass.AP(tensor=t1.tensor, offset=t1[:].offset,
                                               ap=[t1[:].ap[0], [64, nb // 4], [1, 32]]),
                        in1=bass.AP(tensor=t1.tensor, offset=t1[:].offset + 32,
                                    ap=[t1[:].ap[0], [64, nb // 4], [1, 32]]),
                        op=mybir.AluOpType.add,
                    )
                    out1 = wp.tile([128, nb // 8 * 32], f32, tag="out1")
                    nc.vector.tensor_tensor(
                        out=out1[:].rearrange("p (k f) -> p k f", f=32),
                        in0=bass.AP(tensor=t2.tensor, offset=t2[:].offset,
                                                 ap=[t2[:].ap[0], [64, nb // 8], [1, 32]]),
                        in1=bass.AP(tensor=t2.tensor, offset=t2[:].offset + 32,
                                    ap=[t2[:].ap[0], [64, nb // 8], [1, 32]]),
                        op=mybir.AluOpType.add,
                    )
                    # stage 2: per slot-column one-hot matmul into agg windows
                    for sc in range(nb // 8):
                        col = kc0 // 8 + sc
                        cm = colmeta[col]
                        if cm is None:
                            continue
                        a_col, wins, _ = cm
                        for wv in wins:
                            oh = sp.tile([128, 128], f32, tag="oh")
                            nc.vector.tensor_tensor(
                                out=oh[:],
                                in0=iota_t[:, (wv - a_col) * 128 : (wv - a_col + 1) * 128],
                                in1=_bcast_col(dloc_t[:, col : col + 1], 128),
                                op=mybir.AluOpType.is_equal,
                            )
                            pw = ps2.tile([128, 32], f32, tag="pw")
                            nc.tensor.matmul(
                                out=pw[:], lhsT=oh[:],
                                rhs=out1[:, sc * 32 : (sc + 1) * 32],
                                start=True, stop=True,
                            )
                            nc.vector.tensor_tensor(
                                out=agg_tile[:, wv * 32 : (wv + 1) * 32],
                                in0=agg_tile[:, wv * 32 : (wv + 1) * 32],
                                in1=pw[:], op=mybir.AluOpType.add,
                            )

            # ---- layer 1 ----
            agg1 = apool.tile([128, nwin * 32], f32, tag="agg1")
            with tc.tile_pool(name="ps2a", bufs=6, space="PSUM") as ps2:
                layer(table1, agg1, b1s, ps2)
            # relu -> shard2 -> AllGather -> table2
            h2cm = tc.tile_pool(name="h2p", bufs=1)
            h2pool = h2cm.__enter__()
            h2 = h2pool.tile([128, nwin * 64], f32, tag="h2")
            nc.vector.memset(h2[:], 0.0)
            h2v = bass.AP(tensor=h2.tensor, offset=h2[:].offset,
                          ap=[h2[:].ap[0], [64, nwin], [1, 32]])
            nc.scalar.activation(
                out=h2v, in_=agg1[:].rearrange("p (v f) -> p v f", f=32), func=AF.Relu
            )
            dst2 = bass.AP(tensor=shard2.tensor, offset=0,
                           ap=[[64, 128], [128 * 64, nwin], [1, 64]])
            nc.sync.dma_start(out=dst2, in_=h2[:].rearrange("p (v f) -> p v f", f=64))
            h2cm.__exit__(None, None, None)
            nc.gpsimd.collective_compute(
                "AllGather", mybir.AluOpType.bypass,
                ins=[shard2.opt()], outs=[table2.opt()],
                replica_groups=[list(range(CORES))],
            )

            # ---- layer 2 ----
            agg2 = apool.tile([128, nwin * 32], f32, tag="agg1")
            with tc.tile_pool(name="ps2b", bufs=6, space="PSUM") as ps2:
                layer(table2, agg2, None, ps2)

            # ---- out = log_softmax(agg2 @ W2 + b2) ----
            zall = apool.tile([128, nwin * F_OUT], f32, tag="zall")
            sall = apool.tile([128, nwin], f32, tag="sall")
            pf_cm = tc.tile_pool(name="pf", bufs=2, space="PSUM")
            pf = pf_cm.__enter__()
            for v in range(nwin):
                tp = pf.tile([F_HID, 128], f32, tag="tp")
                nc.tensor.transpose(
                    out=tp[:], in_=agg2[:, v * 32 : (v + 1) * 32], identity=id_s[:]
                )
                aT = sp.tile([F_HID, 128], f32, tag="aT")
                nc.vector.tensor_copy(out=aT[:], in_=tp[:])
                zp = pf.tile([128, F_OUT], f32, tag="zp")
                nc.tensor.matmul(out=zp[:], lhsT=aT[:], rhs=W2s[:], start=True, stop=False)
                nc.tensor.matmul(out=zp[:], lhsT=ones_s[:], rhs=b2s[:], start=False, stop=True)
                negm = sp.tile([128, 1], f32, tag="negm")
                nc.vector.reduce_max(out=negm[:], in_=zp[:], axis=AX, negate=True)
                nc.vector.tensor_tensor(
                    out=zall[:, v * F_OUT : (v + 1) * F_OUT],
                    in0=zp[:], in1=_bcast_col(negm[:], F_OUT),
                    op=mybir.AluOpType.add,
                )
                etmp = sp.tile([128, F_OUT], f32, tag="etmp")
                nc.scalar.activation(
                    out=etmp[:], in_=zall[:, v * F_OUT : (v + 1) * F_OUT],
                    func=AF.Exp, accum_out=sall[:, v : v + 1],
                )
            lns = apool.tile([128, nwin], f32, tag="lns")
            nc.scalar.activation(out=lns[:], in_=sall[:], func=AF.Ln)
            for v in range(nwin):
                nc.vector.tensor_tensor(
                    out=zall[:, v * F_OUT : (v + 1) * F_OUT],
                    in0=zall[:, v * F_OUT : (v + 1) * F_OUT],
                    in1=_bcast_col(lns[:, v : v + 1], F_OUT),
                    op=mybir.AluOpType.subtract,
                )
            outdst = bass.AP(
                tensor=out_t, offset=0,
                ap=[[F_OUT, 128], [128 * F_OUT, nwin], [1, F_OUT]],
            )
            nc.sync.dma_start(out=outdst, in_=zall[:].rearrange("p (v f) -> p v f", f=F_OUT))
            pf_cm.__exit__(None, None, None)

    nc.compile()
    return nc


def make_inmaps(meta, inmaps_edges, x, W1, b1, W2, b2):
    geo: Geo = meta["geo"]
    nsh = geo.nsh
    n = geo.n_nodes
    xT_full = np.zeros((F_IN, geo.ntab), np.float32)
    xT_full[:, :n] = np.asarray(x, np.float32).T
    iota = np.tile(np.arange(512, dtype=np.float32)[None, :], (128, 1))
    ident = np.eye(128, dtype=np.float32)
    b1b = np.tile(np.asarray(b1, np.float32)[None, :], (128, 1))
    consts = dict(
        iota512=iota, ident=ident,
        W1t=np.asarray(W1, np.float32), b1t=b1b,
        W2t=np.asarray(W2, np.float32), b2t=np.asarray(b2, np.float32)[None, :],
        onest=np.ones((1, 128), np.float32),
    )
    maps = []
    for c in range(CORES):
        m = dict(inmaps_edges[c])
        m.update(consts)
        m["xT"] = np.ascontiguousarray(xT_full[:, c * nsh : (c + 1) * nsh])
        maps.append(m)
    return maps


_CACHE = {}


def run(x, edge_index, edge_weight, W1, b1, W2, b2, geo=FULL, trace=False):
    key = "geo%d" % geo.n_nodes
    meta, inmaps_edges = pack(edge_index, edge_weight, geo)
    if key in _CACHE:
        nc = _CACHE[key]
    else:
        nc = build(meta)
        _CACHE[key] = nc
    maps = make_inmaps(meta, inmaps_edges, x, W1, b1, W2, b2)
    res = run_bass_kernel_spmd(nc, maps, core_ids=list(range(CORES)), trace=trace)
    n = geo.n_nodes
    out = np.empty((n, F_OUT), np.float32)
    for c in range(CORES):
        lo = c * geo.nsh
        hi = min(lo + geo.nsh, n)
        if hi > lo:
            out[lo:hi] = res.results[c]["out"][: hi - lo]
    return out, res


def kernel(x, edge_index, edge_weight, W1, b1, W2, b2):
    out, _ = run(
        np.asarray(x), np.asarray(edge_index), np.asarray(edge_weight),
        np.asarray(W1), np.asarray(b1), np.asarray(W2), np.asarray(b2),
    )
    return out



# revision 6
# speedup vs baseline: 2.2470x; 2.2470x over previous
"""GCN (2-layer, edge-weighted, log_softmax) on 8 Trainium2 NeuronCores.

Strategy v2 (dst-sharded edges, matmul-based segment-sum, 4-row gather):
  - Nodes sharded 12544/core. Feature tables stored dense node-major
    [100352, 32] f32, viewed as [25088, 128] (4 nodes per 512-B row).
  - Layer k: h = x @ Wk computed data-parallel on node shards -> AllGather
    into the replicated dense table in HBM.
  - Per-edge gather via InstDMAGatherAnt with idx = src//4 (int16 fits
    without group-splitting the table), elem 512 B = the 4-node row; the
    1-of-4 row select folds into the weight multiply (w4 mask grid).
  - Edges packed into 8-edge same-destination slots; DVE applies the w4
    mask-mult and a 4->1 + 8->1 tree reduction; a per-column one-hot
    (is_equal vs iota) matmul segment-sums slot partials into PSUM windows
    of 128 destinations, accumulated into SBUF agg [d%128, (d//128)*32+f].
  - 2048-idx gather instructions keep the SWDGE generation (8 ns/idx, the
    bottleneck) back-to-back while descriptor drains pipeline behind it.
  - Layer 2 aggregates relu(agg1 + b1) with the identical edge structure,
    then applies W2 (+b2) per 128-node window and an on-chip log_softmax.
Host side only packs indices/weights/slot metadata (numpy).
"""

import os
import sys

for _p in ("/opt/trn_rl_repo", "/root/.axon_site/_ro/trn_rl_repo"):
    if os.path.isdir(_p) and _p not in sys.path:
        sys.path.insert(0, _p)

import numpy as np

import concourse.ap_utils as ap_utils
import concourse.bass as bass
import concourse.mybir as mybir
from concourse import bacc, tile
from concourse.bass_utils import run_bass_kernel_spmd

CORES = 8
F_IN = 128
F_HID = 32
F_OUT = 40
KSLOT = 8       # edges per slot (same destination)
GK = 16         # idx-columns per gather instruction (16*128 = 2048 idx)


class Geo:
    def __init__(self, n_nodes=100000, nsh=12544):
        self.n_nodes = n_nodes
        self.nsh = nsh                    # nodes per core shard (mult of 512)
        self.ntab = nsh * CORES           # table nodes (100352)
        self.ngrp = self.ntab // 4        # 4-node gather rows (25088)
        assert nsh % 128 == 0 and nsh % 4 == 0
        self.nwin = nsh // 128            # 128-destination windows per core


FULL = Geo()


def _wrap16(flat, T):
    """token i -> [i%16, i//16], replicated to 128 partitions."""
    a = flat.reshape(T // 16, 16).T
    return np.tile(a, (8, 1)).copy()


def pack(edge_index, edge_weight, geo: Geo):
    """Group edges by (core, dst) into 8-edge slots; build the shared
    column->window template and all per-core device arrays."""
    src = np.asarray(edge_index[0], dtype=np.int64)
    dst = np.asarray(edge_index[1], dtype=np.int64)
    w = np.asarray(edge_weight, dtype=np.float32)
    nsh, nwin = geo.nsh, geo.nwin

    core = dst // nsh
    pc = []
    cnt = np.zeros((CORES, nwin), np.int64)
    for c in range(CORES):
        m = core == c
        s_c = src[m]
        dl = dst[m] - c * nsh
        wc = w[m]
        order = np.argsort(dl, kind="stable")
        sk = s_c[order]
        dlk = dl[order]
        wk = wc[order]
        new = np.r_[True, dlk[1:] != dlk[:-1]]
        run_first = np.flatnonzero(new)
        run_len = np.diff(np.r_[run_first, len(dlk)])
        run_id = np.cumsum(new) - 1
        rank = np.arange(len(dlk)) - run_first[run_id]
        nsl = (run_len + KSLOT - 1) // KSLOT
        dl_run = dlk[run_first]
        v_run = dl_run // 128
        np.add.at(cnt[c], v_run, nsl)
        pc.append((sk, dlk, wk, rank, run_id, nsl, dl_run, v_run))

    # shared template: per-window slot capacity = max over cores, round to 8
    cap = cnt.max(axis=0)
    cap = ((cap + 7) // 8) * 8
    off = np.zeros(nwin, np.int64)
    b = 0
    for v in range(nwin):
        off[v] = b
        b += cap[v]
    S_T = int(((b + 255) // 256) * 256)   # slots; T = 8*S_T mult of 2048
    COLS = S_T // 128
    T = S_T * KSLOT
    KC = T // 128                          # idx-columns total

    # per-column window lists (template)
    colmeta = []
    flat_off = np.r_[off, S_T]
    for col in range(COLS):
        lo, hi = col * 128, col * 128 + 128
        i0 = int(np.searchsorted(flat_off, lo, side="right") - 1)
        wins = []
        for v in range(max(i0, 0), nwin):
            if flat_off[v] >= hi:
                break
            if flat_off[v + 1] <= lo:
                continue
            wins.append(v)
        colmeta.append((wins[0], wins) if wins else None)

    ginstr = []
    kc = 0
    while kc < KC:
        ginstr.append((kc, min(GK, KC - kc)))
        kc += GK

    # per-core arrays
    inmaps = []
    for c in range(CORES):
        sk, dlk, wk, rank, run_id, nsl, dl_run, v_run = pc[c]
        csum = np.cumsum(nsl)
        start_excl = np.r_[0, csum[:-1]]
        newv = np.r_[True, v_run[1:] != v_run[:-1]]
        v_first = np.flatnonzero(newv)
        v_id = np.cumsum(newv) - 1
        base_in_v = start_excl - start_excl[v_first][v_id]
        run_slot = off[v_run] + base_in_v
        slot_e = run_slot[run_id] + rank // KSLOT
        j_e = rank % KSLOT
        cs_e = slot_e // 128
        p_e = slot_e % 128
        tok = (cs_e * KSLOT + j_e) * 128 + p_e

        idx_flat = np.zeros(T, np.int16)
        idx_flat[tok] = (sk // 4).astype(np.int16)
        w4_flat = np.zeros((T, 4), np.float32)
        w4_flat[tok, sk % 4] = wk

        n_runs = len(nsl)
        dl_slot = np.full(S_T, float(geo.n_nodes), np.float32)
        reps = np.repeat(np.arange(n_runs), nsl)
        ar = np.arange(len(reps)) - np.repeat(start_excl, nsl)
        pos = np.repeat(run_slot, nsl) + ar
        dl_slot[pos] = np.repeat(dl_run, nsl).astype(np.float32)
        dcol = dl_slot.reshape(COLS, 128).T.copy()
        for col in range(COLS):
            if colmeta[col] is not None:
                dcol[:, col] -= 128.0 * colmeta[col][0]

        inmaps.append(
            dict(
                gidx=_wrap16(idx_flat, T),
                w4g=w4_flat.reshape(KC, 128, 4).transpose(1, 0, 2).reshape(128, KC * 4).copy(),
                dloc=dcol,
            )
        )

    meta = dict(S_T=S_T, COLS=COLS, T=T, KC=KC, colmeta=colmeta, ginstr=ginstr, geo=geo)
    return meta, inmaps


def emit_dma_gather(gp, out_ap, in_ap, idxs_ap, num_idxs, elem_size, elem_step,
                    single_packet=False):
    """bass.dma_gather minus the blanket 256B elem assert."""
    from concourse.bass import exact_div

    assert idxs_ap.dtype == mybir.dt.int16
    assert in_ap.dtype == out_ap.dtype
    assert in_ap.space == bass.MemorySpace.DRAM
    assert ap_utils.ap_is_contiguous(in_ap.ap[1:])
    assert ap_utils.ap_is_contiguous(out_ap.ap[1:])
    assert ap_utils.ap_is_contiguous(idxs_ap.ap[1:])
    assert in_ap.ap[-1][1] == out_ap.ap[-1][1] == elem_size
    assert out_ap.ap[0][1] * out_ap.ap[1][1] == num_idxs
    assert in_ap.ap[0][0] == elem_step
    stride_bytes_256 = exact_div(elem_step * mybir.dt.size(in_ap.dtype), 256)
    assert stride_bytes_256 < 256
    _in_ap = gp.lower_ap_dma(in_ap, for_custom_bir_dma=True)
    _idxs_ap = gp.lower_ap(idxs_ap)
    _out_ap = gp.lower_ap(out_ap)
    return gp.add_instruction(
        mybir.InstDMAGatherAnt(
            name=gp.bass.get_next_instruction_name(),
            ins=[*_in_ap, _idxs_ap, gp.lower_val_access(gp.to_reg(num_idxs))],
            outs=[_out_ap],
            transpose=False,
            num_idxs=num_idxs,
            elem_size=elem_size,
            stride_bytes_256=stride_bytes_256,
            gen_mode=0,
            single_packet=single_packet,
            queue_num=0,
            sbuf_tokens_per_rank=0,
            sbuf_free_dim_per_rank=0,
            sbuf_free_dim_pad_per_rank=0,
            sbuf_byte_offset=0,
        )
    )


def _b(ap2, reps):
    """broadcast each element of an AP over `reps` trailing copies."""
    return bass.AP(tensor=ap2.tensor, offset=ap2.offset, ap=[*ap2.ap, [0, reps]])


def _bcast_col(ap1, n):
    """[P, 1] AP -> [P, n] zero-stride broadcast."""
    return bass.AP(tensor=ap1.tensor, offset=ap1.offset, ap=[ap1.ap[0], [0, n]])


def build(meta):
    geo: Geo = meta["geo"]
    S_T, COLS, T, KC = meta["S_T"], meta["COLS"], meta["T"], meta["KC"]
    colmeta, ginstr = meta["colmeta"], meta["ginstr"]
    nsh, ngrp, nwin = geo.nsh, geo.ngrp, geo.nwin
    f32 = mybir.dt.float32
    AX = mybir.AxisListType.X
    AF = mybir.ActivationFunctionType

    nc = bacc.Bacc("TRN2", target_bir_lowering=False, debug=False, num_devices=CORES)

    xT = nc.dram_tensor("xT", [F_IN, nsh], f32, kind="ExternalInput")
    gidx = nc.dram_tensor("gidx", [128, T // 16], mybir.dt.int16, kind="ExternalInput")
    w4g = nc.dram_tensor("w4g", [128, KC * 4], f32, kind="ExternalInput")
    dloc = nc.dram_tensor("dloc", [128, COLS], f32, kind="ExternalInput")
    iota512 = nc.dram_tensor("iota512", [128, 512], f32, kind="ExternalInput")
    ident = nc.dram_tensor("ident", [128, 128], f32, kind="ExternalInput")
    W1t = nc.dram_tensor("W1t", [F_IN, F_HID], f32, kind="ExternalInput")
    b1t = nc.dram_tensor("b1t", [128, F_HID], f32, kind="ExternalInput")
    W2t = nc.dram_tensor("W2t", [F_HID, F_OUT], f32, kind="ExternalInput")
    b2t = nc.dram_tensor("b2t", [1, F_OUT], f32, kind="ExternalInput")
    onest = nc.dram_tensor("onest", [1, 128], f32, kind="ExternalInput")
    out_t = nc.dram_tensor("out", [nsh, F_OUT], f32, kind="ExternalOutput")

    with tile.TileContext(nc) as tc:
        with (
            tc.tile_pool(name="const", bufs=1) as cpool,
            tc.tile_pool(name="dram", bufs=1, space="DRAM") as dram,
            tc.tile_pool(name="gxp", bufs=4) as gxp,
            tc.tile_pool(name="work", bufs=4) as wp,
            tc.tile_pool(name="scol", bufs=8) as sp,
            tc.tile_pool(name="agg", bufs=1) as apool,
        ):
            iota_t = cpool.tile([128, 512], f32)
            nc.sync.dma_start(out=iota_t[:], in_=iota512[:, :])
            dloc_t = cpool.tile([128, COLS], f32)
            nc.sync.dma_start(out=dloc_t[:], in_=dloc[:, :])
            W1s = cpool.tile([F_IN, F_HID], f32)
            nc.sync.dma_start(out=W1s[:], in_=W1t[:, :])
            b1s = cpool.tile([128, F_HID], f32)
            nc.sync.dma_start(out=b1s[:], in_=b1t[:, :])
            W2s = cpool.tile([F_HID, F_OUT], f32)
            nc.sync.dma_start(out=W2s[:], in_=W2t[:, :])
            b2s = cpool.tile([1, F_OUT], f32)
            nc.sync.dma_start(out=b2s[:], in_=b2t[:, :])
            ones_s = cpool.tile([1, 128], f32)
            nc.sync.dma_start(out=ones_s[:], in_=onest[:, :])
            id_s = cpool.tile([128, 128], f32)
            nc.sync.dma_start(out=id_s[:], in_=ident[:, :])

            shard1 = dram.tile([nsh // 4, 128], f32)
            shard2 = dram.tile([nsh // 4, 128], f32)
            table1 = dram.tile([ngrp, 128], f32)
            table2 = dram.tile([ngrp, 128], f32)

            # ---- h = x @ W1 on own shard -> shard1 (dense [nsh, 32]) ----
            with (
                tc.tile_pool(name="xt", bufs=1) as xp,
                tc.tile_pool(name="ph", bufs=2, space="PSUM") as ph,
            ):
                half = nsh // 2
                for hh in range(2):
                    xTs = xp.tile([F_IN, half], f32, tag="xts")
                    nc.sync.dma_start(out=xTs[:], in_=xT[:, hh * half : (hh + 1) * half])
                    for tt in range(half // 128):
                        t = hh * (half // 128) + tt
                        hp = ph.tile([128, F_HID], f32, tag="hps")
                        nc.tensor.matmul(
                            out=hp[:], lhsT=xTs[:, tt * 128 : (tt + 1) * 128], rhs=W1s[:],
                            start=True, stop=True,
                        )
                        hs = wp.tile([128, F_HID], f32, tag="hsb")
                        nc.vector.tensor_copy(out=hs[:], in_=hp[:])
                        dst = bass.AP(
                            tensor=shard1.tensor, offset=t * 128 * 32,
                            ap=[[32, 128], [1, 32]],
                        )
                        nc.sync.dma_start(out=dst, in_=hs[:])

            nc.gpsimd.collective_compute(
                "AllGather", mybir.AluOpType.bypass,
                ins=[shard1.opt()], outs=[table1.opt()],
                replica_groups=[list(range(CORES))],
            )

            def layer(table, agg_tile, init_b, ps2):
                if init_b is not None:
                    bsrc = bass.AP(
                        tensor=init_b.tensor, offset=init_b[:].offset,
                        ap=[init_b[:].ap[0], [0, nwin], [1, 32]],
                    )
                    nc.vector.tensor_copy(
                        out=agg_tile[:].rearrange("p (v f) -> p v f", f=32), in_=bsrc
                    )
                else:
                    nc.vector.memset(agg_tile[:], 0.0)
                for kc0, gk in ginstr:
                    nid = gk * 128          # idx per instruction
                    gx = gxp.tile([128, gk * 8], mybir.dt.int16, tag="gx")
                    nc.sync.dma_start(out=gx[:], in_=gidx[:, kc0 * 8 : (kc0 + gk) * 8])
                    w4 = gxp.tile([128, gk * 4], f32, tag="w4")
                    nc.sync.dma_start(out=w4[:], in_=w4g[:, kc0 * 4 : (kc0 + gk) * 4])
                    msgs = wp.tile([128, gk * 128], f32, tag="msgs")
                    emit_dma_gather(
                        nc.gpsimd,
                        out_ap=msgs[:].rearrange("p (k f) -> p k f", f=128),
                        in_ap=bass.AP(
                            tensor=table.tensor, offset=0,
                            ap=[[128, ngrp], [1, 128]],
                        ),
                        idxs_ap=gx[:],
                        num_idxs=nid,
                        elem_size=128,
                        elem_step=128,
                    )
                    # mask-weight multiply: [p, k*4, 32] *= w4 bcast over 32
                    nc.vector.tensor_tensor(
                        out=msgs[:].rearrange("p (q f) -> p q f", f=32),
                        in0=msgs[:].rearrange("p (q f) -> p q f", f=32),
                        in1=_b(w4[:], 32), op=mybir.AluOpType.mult,
                    )
                    def pairsum(dst_tile, src_tile, nblk):
                        # dst[i] = src[2i] + src[2i+1] over nblk 32-f blocks
                        nc.vector.tensor_tensor(
                            out=dst_tile[:].rearrange("p (q f) -> p q f", f=32),
                            in0=bass.AP(tensor=src_tile.tensor, offset=src_tile[:].offset,
                                        ap=[src_tile[:].ap[0], [64, nblk], [1, 32]]),
                            in1=bass.AP(tensor=src_tile.tensor, offset=src_tile[:].offset + 32,
                                        ap=[src_tile[:].ap[0], [64, nblk], [1, 32]]),
                            op=mybir.AluOpType.add,
                        )
                    s2 = wp.tile([128, gk * 64], f32, tag="s2")
                    pairsum(s2, msgs, gk * 2)
                    s1 = wp.tile([128, gk * 32], f32, tag="s1")
                    pairsum(s1, s2, gk)
                    t1 = wp.tile([128, gk * 16], f32, tag="t1")
                    pairsum(t1, s1, gk // 2)
                    t2 = wp.tile([128, gk * 8], f32, tag="t2")
                    pairsum(t2, t1, gk // 4)
                    out1 = wp.tile([128, gk * 4], f32, tag="out1")
                    pairsum(out1, t2, gk // 8)
                    # stage 2: per slot-column one-hot matmul into agg windows
                    for sc in range(gk // 8):
                        col = kc0 // 8 + sc
                        cm = colmeta[col]
                        if cm is None:
                            continue
                        a_col, wins = cm
                        for wv in wins:
                            oh = sp.tile([128, 128], f32, tag="oh")
                            nc.vector.tensor_tensor(
                                out=oh[:],
                                in0=iota_t[:, (wv - a_col) * 128 : (wv - a_col + 1) * 128],
                                in1=_bcast_col(dloc_t[:, col : col + 1], 128),
                                op=mybir.AluOpType.is_equal,
                            )
                            pw = ps2.tile([128, 32], f32, tag="pw")
                            nc.tensor.matmul(
                                out=pw[:], lhsT=oh[:],
                                rhs=out1[:, sc * 32 : (sc + 1) * 32],
                                start=True, stop=True,
                            )
                            nc.vector.tensor_tensor(
                                out=agg_tile[:, wv * 32 : (wv + 1) * 32],
                                in0=agg_tile[:, wv * 32 : (wv + 1) * 32],
                                in1=pw[:], op=mybir.AluOpType.add,
                            )

            # ---- layer 1 ----
            agg1 = apool.tile([128, nwin * 32], f32, tag="agg1")
            with tc.tile_pool(name="ps2a", bufs=6, space="PSUM") as ps2:
                layer(table1, agg1, b1s, ps2)
            # relu -> shard2 (dense [nsh, 32]) -> AllGather -> table2
            h2cm = tc.tile_pool(name="h2p", bufs=1)
            h2pool = h2cm.__enter__()
            h2 = h2pool.tile([128, nwin * 32], f32, tag="h2")
            nc.scalar.activation(
                out=h2[:].rearrange("p (v f) -> p v f", f=32),
                in_=agg1[:].rearrange("p (v f) -> p v f", f=32), func=AF.Relu,
            )
            dst2 = bass.AP(tensor=shard2.tensor, offset=0,
                           ap=[[32, 128], [128 * 32, nwin], [1, 32]])
            nc.sync.dma_start(out=dst2, in_=h2[:].rearrange("p (v f) -> p v f", f=32))
            h2cm.__exit__(None, None, None)
            nc.gpsimd.collective_compute(
                "AllGather", mybir.AluOpType.bypass,
                ins=[shard2.opt()], outs=[table2.opt()],
                replica_groups=[list(range(CORES))],
            )

            # ---- layer 2 ----
            agg2 = apool.tile([128, nwin * 32], f32, tag="agg1")
            with tc.tile_pool(name="ps2b", bufs=6, space="PSUM") as ps2:
                layer(table2, agg2, None, ps2)

            # ---- out = log_softmax(agg2 @ W2 + b2) ----
            zall = apool.tile([128, nwin * F_OUT], f32, tag="zall")
            sall = apool.tile([128, nwin], f32, tag="sall")
            pf_cm = tc.tile_pool(name="pf", bufs=2, space="PSUM")
            pf = pf_cm.__enter__()
            for v in range(nwin):
                tp = pf.tile([F_HID, 128], f32, tag="tp")
                nc.tensor.transpose(
                    out=tp[:], in_=agg2[:, v * 32 : (v + 1) * 32], identity=id_s[:]
                )
                aT = sp.tile([F_HID, 128], f32, tag="aT")
                nc.vector.tensor_copy(out=aT[:], in_=tp[:])
                zp = pf.tile([128, F_OUT], f32, tag="zp")
                nc.tensor.matmul(out=zp[:], lhsT=aT[:], rhs=W2s[:], start=True, stop=False)
                nc.tensor.matmul(out=zp[:], lhsT=ones_s[:], rhs=b2s[:], start=False, stop=True)
                negm = sp.tile([128, 1], f32, tag="negm")
                nc.vector.reduce_max(out=negm[:], in_=zp[:], axis=AX, negate=True)
                nc.vector.tensor_tensor(
                    out=zall[:, v * F_OUT : (v + 1) * F_OUT],
                    in0=zp[:], in1=_bcast_col(negm[:], F_OUT),
                    op=mybir.AluOpType.add,
                )
                etmp = sp.tile([128, F_OUT], f32, tag="etmp")
                nc.scalar.activation(
                    out=etmp[:], in_=zall[:, v * F_OUT : (v + 1) * F_OUT],
                    func=AF.Exp, accum_out=sall[:, v : v + 1],
                )
            lns = apool.tile([128, nwin], f32, tag="lns")
            nc.scalar.activation(out=lns[:], in_=sall[:], func=AF.Ln)
            for v in range(nwin):
                nc.vector.tensor_tensor(
                    out=zall[:, v * F_OUT : (v + 1) * F_OUT],
                    in0=zall[:, v * F_OUT : (v + 1) * F_OUT],
                    in1=_bcast_col(lns[:, v : v + 1], F_OUT),
                    op=mybir.AluOpType.subtract,
                )
            outdst = bass.AP(
                tensor=out_t, offset=0,
                ap=[[F_OUT, 128], [128 * F_OUT, nwin], [1, F_OUT]],
            )
            nc.sync.dma_start(out=outdst, in_=zall[:].rearrange("p (v f) -> p v f", f=F_OUT))
            pf_cm.__exit__(None, None, None)

    nc.compile()
    return nc


def make_inmaps(meta, inmaps_edges, x, W1, b1, W2, b2):
    geo: Geo = meta["geo"]
    nsh = geo.nsh
    n = geo.n_nodes
    xT_full = np.zeros((F_IN, geo.ntab), np.float32)
    xT_full[:, :n] = np.asarray(x, np.float32).T
    iota = np.tile(np.arange(512, dtype=np.float32)[None, :], (128, 1))
    ident = np.eye(128, dtype=np.float32)
    b1b = np.tile(np.asarray(b1, np.float32)[None, :], (128, 1))
    consts = dict(
        iota512=iota, ident=ident,
        W1t=np.asarray(W1, np.float32), b1t=b1b,
        W2t=np.asarray(W2, np.float32), b2t=np.asarray(b2, np.float32)[None, :],
        onest=np.ones((1, 128), np.float32),
    )
    maps = []
    for c in range(CORES):
        m = dict(inmaps_edges[c])
        m.update(consts)
        m["xT"] = np.ascontiguousarray(xT_full[:, c * nsh : (c + 1) * nsh])
        maps.append(m)
    return maps


_CACHE = {}


def run(x, edge_index, edge_weight, W1, b1, W2, b2, geo=FULL, trace=False):
    meta, inmaps_edges = pack(edge_index, edge_weight, geo)
    key = ("geo%d" % geo.n_nodes, meta["S_T"])
    if key in _CACHE:
        nc = _CACHE[key]
    else:
        nc = build(meta)
        _CACHE[key] = nc
    maps = make_inmaps(meta, inmaps_edges, x, W1, b1, W2, b2)
    res = run_bass_kernel_spmd(nc, maps, core_ids=list(range(CORES)), trace=trace)
    n = geo.n_nodes
    out = np.empty((n, F_OUT), np.float32)
    for c in range(CORES):
        lo = c * geo.nsh
        hi = min(lo + geo.nsh, n)
        if hi > lo:
            out[lo:hi] = res.results[c]["out"][: hi - lo]
    return out, res


def kernel(x, edge_index, edge_weight, W1, b1, W2, b2):
    out, _ = run(
        np.asarray(x), np.asarray(edge_index), np.asarray(edge_weight),
        np.asarray(W1), np.asarray(b1), np.asarray(W2), np.asarray(b2),
    )
    return out


# revision 13
# speedup vs baseline: 2.3874x; 1.0625x over previous
"""GCN (2-layer, edge-weighted, log_softmax) on 8 Trainium2 NeuronCores.

Strategy v2 (dst-sharded edges, matmul-based segment-sum, 4-row gather):
  - Nodes sharded 12544/core. Feature tables stored dense node-major
    [100352, 32] f32, viewed as [25088, 128] (4 nodes per 512-B row).
  - Layer k: h = x @ Wk computed data-parallel on node shards -> AllGather
    into the replicated dense table in HBM.
  - Per-edge gather via InstDMAGatherAnt with idx = src//4 (int16 fits
    without group-splitting the table), elem 512 B = the 4-node row; the
    1-of-4 row select folds into the weight multiply (w4 mask grid).
  - Edges packed into 8-edge same-destination slots; DVE applies the w4
    mask-mult and a 4->1 + 8->1 tree reduction; a per-column one-hot
    (is_equal vs iota) matmul segment-sums slot partials into PSUM windows
    of 128 destinations, accumulated into SBUF agg [d%128, (d//128)*32+f].
  - 2048-idx gather instructions keep the SWDGE generation (8 ns/idx, the
    bottleneck) back-to-back while descriptor drains pipeline behind it.
  - Layer 2 aggregates relu(agg1 + b1) with the identical edge structure,
    then applies W2 (+b2) per 128-node window and an on-chip log_softmax.
Host side only packs indices/weights/slot metadata (numpy).
"""

import os
import sys

for _p in ("/opt/trn_rl_repo", "/root/.axon_site/_ro/trn_rl_repo"):
    if os.path.isdir(_p) and _p not in sys.path:
        sys.path.insert(0, _p)

import numpy as np

import concourse.ap_utils as ap_utils
import concourse.bass as bass
import concourse.mybir as mybir
from concourse import bacc, tile
from concourse.bass_utils import run_bass_kernel_spmd

CORES = 8
F_IN = 128
F_HID = 32
F_OUT = 40
KSLOT = 8       # edges per slot (same destination)
GK = 16         # idx-columns per gather instruction (16*128 = 2048 idx)


class Geo:
    def __init__(self, n_nodes=100000, nsh=12544):
        self.n_nodes = n_nodes
        self.nsh = nsh                    # nodes per core shard (mult of 512)
        self.ntab = nsh * CORES           # table nodes (100352)
        self.ngrp = self.ntab // 4        # 4-node gather rows (25088)
        assert nsh % 128 == 0 and nsh % 4 == 0
        self.nwin = nsh // 128            # 128-destination windows per core


FULL = Geo()


def _wrap16(flat, T):
    """token i -> [i%16, i//16], replicated to 128 partitions."""
    a = flat.reshape(T // 16, 16).T
    return np.tile(a, (8, 1)).copy()


TIERS = (8, 4, 2, 1)


def pack(edge_index, edge_weight, geo: Geo):
    """Group edges by (core, dst) into same-destination slots of size 8/4/2/1
    (binary decomposition of each run length); build the shared
    column->window template per tier and all per-core device arrays."""
    src = np.asarray(edge_index[0], dtype=np.int64)
    dst = np.asarray(edge_index[1], dtype=np.int64)
    w = np.asarray(edge_weight, dtype=np.float32)
    nsh, nwin = geo.nsh, geo.nwin

    core = dst // nsh
    pc = []
    cnt = {t: np.zeros((CORES, nwin), np.int64) for t in TIERS}
    for c in range(CORES):
        m = core == c
        order = np.argsort(dst[m], kind="stable")
        sk = src[m][order]
        dlk = (dst[m] - c * nsh)[order]
        wk = w[m][order]
        new = np.r_[True, dlk[1:] != dlk[:-1]]
        run_first = np.flatnonzero(new)
        run_len = np.diff(np.r_[run_first, len(dlk)])
        run_id = np.cumsum(new) - 1
        rank = np.arange(len(dlk)) - run_first[run_id]
        dl_run = dlk[run_first]
        v_run = dl_run // 128
        # per-run slot counts per tier (binary decomposition)
        n_t = {8: run_len // 8, 4: (run_len % 8) // 4,
               2: (run_len % 4) // 2, 1: run_len % 2}
        for t in TIERS:
            np.add.at(cnt[t][c], v_run, n_t[t])
        pc.append((sk, dlk, wk, rank, run_id, run_len, dl_run, v_run, n_t))

    # shared template per tier: window capacity = max over cores, round to 8;
    # section slot count S_t padded so S_t * t is a multiple of 2048 tokens.
    off, S, CB, KB, TB = {}, {}, {}, {}, {}
    cols_acc = 0
    kcol_acc = 0
    tok_acc = 0
    colmeta = []
    for t in TIERS:
        cap = cnt[t].max(axis=0)
        cap = ((cap + 7) // 8) * 8
        o = np.zeros(nwin, np.int64)
        b = 0
        for v in range(nwin):
            o[v] = b
            b += cap[v]
        align = 2048 // t
        S_t = int((b + align - 1) // align * align)
        off[t], S[t] = o, S_t
        CB[t], KB[t], TB[t] = cols_acc, kcol_acc, tok_acc
        flat_off = np.r_[o, S_t]
        for col in range(S_t // 128):
            lo, hi = col * 128, col * 128 + 128
            i0 = int(np.searchsorted(flat_off, lo, side="right") - 1)
            wins = []
            for v in range(max(i0, 0), nwin):
                if flat_off[v] >= hi:
                    break
                if flat_off[v + 1] <= lo:
                    continue
                wins.append(v)
            colmeta.append((wins[0], wins) if wins else None)
        cols_acc += S_t // 128
        kcol_acc += S_t * t // 128
        tok_acc += S_t * t
    COLS, KC, T = cols_acc, kcol_acc, tok_acc

    ginstr = []  # (kc0 global, gk, tier)
    for t in TIERS:
        kc0, kc1 = KB[t], KB[t] + S[t] * t // 128
        kc = kc0
        while kc < kc1:
            ginstr.append((kc, min(GK, kc1 - kc), t))
            kc += GK

    # per-core arrays
    inmaps = []
    for c in range(CORES):
        sk, dlk, wk, rank, run_id, run_len, dl_run, v_run, n_t = pc[c]
        idx_flat = np.zeros(T, np.int16)
        w4_flat = np.zeros((T, 4), np.float32)
        dl_slot_all = np.full(COLS * 128, float(geo.n_nodes), np.float32)
        # rank boundaries within each run for tier assignment
        l8 = run_len // 8 * 8
        lo_t = {8: np.zeros(len(run_len), np.int64), 4: l8,
                2: l8 + n_t[4] * 4, 1: l8 + n_t[4] * 4 + n_t[2] * 2}
        for t in TIERS:
            nsl = n_t[t]
            csum = np.cumsum(nsl)
            start_excl = np.r_[0, csum[:-1]]
            newv = np.r_[True, v_run[1:] != v_run[:-1]]
            v_first = np.flatnonzero(newv)
            v_id = np.cumsum(newv) - 1
            base_in_v = start_excl - start_excl[v_first][v_id]
            run_slot = off[t][v_run] + base_in_v  # slot within tier section
            # edges of this tier: rank in [lo_t[t][run], lo_t[t][run]+nsl*t)
            rr = rank - lo_t[t][run_id]
            sel = (rr >= 0) & (rr < nsl[run_id] * t)
            rsel = rr[sel]
            rid = run_id[sel]
            slot_e = run_slot[rid] + rsel // t
            j_e = rsel % t
            tok = TB[t] + (slot_e // 128 * t + j_e) * 128 + slot_e % 128
            idx_flat[tok] = (sk[sel] // 4).astype(np.int16)
            w4_flat[tok, sk[sel] % 4] = wk[sel]
            # slot dst values for this tier's columns
            n_runs = len(nsl)
            reps = np.repeat(np.arange(n_runs), nsl)
            ar = np.arange(len(reps)) - np.repeat(start_excl, nsl)
            pos = np.repeat(run_slot, nsl) + ar
            dl_slot_all[CB[t] * 128 + pos] = np.repeat(dl_run, nsl).astype(np.float32)
        dcol = dl_slot_all.reshape(COLS, 128).T.copy()
        for col in range(COLS):
            if colmeta[col] is not None:
                dcol[:, col] -= 128.0 * colmeta[col][0]

        inmaps.append(
            dict(
                gidx=_wrap16(idx_flat, T),
                w4g=w4_flat.reshape(KC, 128, 4).transpose(1, 0, 2).reshape(128, KC * 4).copy(),
                dloc=dcol,
            )
        )

    meta = dict(S_T=COLS * 128, COLS=COLS, T=T, KC=KC, colmeta=colmeta,
                ginstr=ginstr, CB=CB, KB=KB, geo=geo)
    return meta, inmaps


def emit_dma_gather(gp, out_ap, in_ap, idxs_ap, num_idxs, elem_size, elem_step,
                    single_packet=False):
    """bass.dma_gather minus the blanket 256B elem assert."""
    from concourse.bass import exact_div

    assert idxs_ap.dtype == mybir.dt.int16
    assert in_ap.dtype == out_ap.dtype
    assert in_ap.space == bass.MemorySpace.DRAM
    assert ap_utils.ap_is_contiguous(in_ap.ap[1:])
    assert ap_utils.ap_is_contiguous(out_ap.ap[1:])
    assert ap_utils.ap_is_contiguous(idxs_ap.ap[1:])
    assert in_ap.ap[-1][1] == out_ap.ap[-1][1] == elem_size
    assert out_ap.ap[0][1] * out_ap.ap[1][1] == num_idxs
    assert in_ap.ap[0][0] == elem_step
    stride_bytes_256 = exact_div(elem_step * mybir.dt.size(in_ap.dtype), 256)
    assert stride_bytes_256 < 256
    _in_ap = gp.lower_ap_dma(in_ap, for_custom_bir_dma=True)
    _idxs_ap = gp.lower_ap(idxs_ap)
    _out_ap = gp.lower_ap(out_ap)
    return gp.add_instruction(
        mybir.InstDMAGatherAnt(
            name=gp.bass.get_next_instruction_name(),
            ins=[*_in_ap, _idxs_ap, gp.lower_val_access(gp.to_reg(num_idxs))],
            outs=[_out_ap],
            transpose=False,
            num_idxs=num_idxs,
            elem_size=elem_size,
            stride_bytes_256=stride_bytes_256,
            gen_mode=0,
            single_packet=single_packet,
            queue_num=0,
            sbuf_tokens_per_rank=0,
            sbuf_free_dim_per_rank=0,
            sbuf_free_dim_pad_per_rank=0,
            sbuf_byte_offset=0,
        )
    )


def _b(ap2, reps):
    """broadcast each element of an AP over `reps` trailing copies."""
    return bass.AP(tensor=ap2.tensor, offset=ap2.offset, ap=[*ap2.ap, [0, reps]])


def _bcast_col(ap1, n):
    """[P, 1] AP -> [P, n] zero-stride broadcast."""
    return bass.AP(tensor=ap1.tensor, offset=ap1.offset, ap=[ap1.ap[0], [0, n]])


def build(meta):
    geo: Geo = meta["geo"]
    S_T, COLS, T, KC = meta["S_T"], meta["COLS"], meta["T"], meta["KC"]
    colmeta, ginstr = meta["colmeta"], meta["ginstr"]
    CB, KB = meta["CB"], meta["KB"]
    nsh, ngrp, nwin = geo.nsh, geo.ngrp, geo.nwin
    f32 = mybir.dt.float32
    AX = mybir.AxisListType.X
    AF = mybir.ActivationFunctionType

    nc = bacc.Bacc("TRN2", target_bir_lowering=False, debug=False, num_devices=CORES)

    xT = nc.dram_tensor("xT", [F_IN, nsh], f32, kind="ExternalInput")
    gidx = nc.dram_tensor("gidx", [128, T // 16], mybir.dt.int16, kind="ExternalInput")
    w4g = nc.dram_tensor("w4g", [128, KC * 4], f32, kind="ExternalInput")
    dloc = nc.dram_tensor("dloc", [128, COLS], f32, kind="ExternalInput")
    iota512 = nc.dram_tensor("iota512", [128, 512], f32, kind="ExternalInput")
    ident = nc.dram_tensor("ident", [128, 128], f32, kind="ExternalInput")
    W1t = nc.dram_tensor("W1t", [F_IN, F_HID], f32, kind="ExternalInput")
    b1t = nc.dram_tensor("b1t", [128, F_HID], f32, kind="ExternalInput")
    W2t = nc.dram_tensor("W2t", [F_HID, F_OUT], f32, kind="ExternalInput")
    b2t = nc.dram_tensor("b2t", [1, F_OUT], f32, kind="ExternalInput")
    onest = nc.dram_tensor("onest", [1, 128], f32, kind="ExternalInput")
    out_t = nc.dram_tensor("out", [nsh, F_OUT], f32, kind="ExternalOutput")

    with tile.TileContext(nc) as tc:
        with (
            tc.tile_pool(name="const", bufs=1) as cpool,
            tc.tile_pool(name="dram", bufs=1, space="DRAM") as dram,
            tc.tile_pool(name="gxp", bufs=4) as gxp,
            tc.tile_pool(name="work", bufs=4) as wp,
            tc.tile_pool(name="scol", bufs=8) as sp,
            tc.tile_pool(name="agg", bufs=1) as apool,
        ):
            iota_t = cpool.tile([128, 512], f32)
            nc.sync.dma_start(out=iota_t[:], in_=iota512[:, :])
            dloc_t = cpool.tile([128, COLS], f32)
            nc.sync.dma_start(out=dloc_t[:], in_=dloc[:, :])
            W1s = cpool.tile([F_IN, F_HID], f32)
            nc.sync.dma_start(out=W1s[:], in_=W1t[:, :])
            b1s = cpool.tile([128, F_HID], f32)
            nc.sync.dma_start(out=b1s[:], in_=b1t[:, :])
            W2s = cpool.tile([F_HID, F_OUT], f32)
            nc.sync.dma_start(out=W2s[:], in_=W2t[:, :])
            b2s = cpool.tile([1, F_OUT], f32)
            nc.sync.dma_start(out=b2s[:], in_=b2t[:, :])
            ones_s = cpool.tile([1, 128], f32)
            nc.sync.dma_start(out=ones_s[:], in_=onest[:, :])
            id_s = cpool.tile([128, 128], f32)
            nc.sync.dma_start(out=id_s[:], in_=ident[:, :])

            shard1 = dram.tile([nsh // 4, 128], f32)
            shard2 = dram.tile([nsh // 4, 128], f32)
            table1 = dram.tile([ngrp, 128], f32, addr_space="Shared")
            table2 = dram.tile([ngrp, 128], f32, addr_space="Shared")

            # ---- h = x @ W1 on own shard -> shard1 (dense [nsh, 32]) ----
            with (
                tc.tile_pool(name="xt", bufs=1) as xp,
                tc.tile_pool(name="ph", bufs=2, space="PSUM") as ph,
            ):
                half = nsh // 2
                for hh in range(2):
                    xTs = xp.tile([F_IN, half], f32, tag="xts")
                    nc.sync.dma_start(out=xTs[:], in_=xT[:, hh * half : (hh + 1) * half])
                    for tt in range(half // 128):
                        t = hh * (half // 128) + tt
                        hp = ph.tile([128, F_HID], f32, tag="hps")
                        nc.tensor.matmul(
                            out=hp[:], lhsT=xTs[:, tt * 128 : (tt + 1) * 128], rhs=W1s[:],
                            start=True, stop=True,
                        )
                        hs = wp.tile([128, F_HID], f32, tag="hsb")
                        nc.vector.tensor_copy(out=hs[:], in_=hp[:])
                        dst = bass.AP(
                            tensor=shard1.tensor, offset=t * 128 * 32,
                            ap=[[32, 128], [1, 32]],
                        )
                        nc.sync.dma_start(out=dst, in_=hs[:])

            nc.gpsimd.collective_compute(
                "AllGather", mybir.AluOpType.bypass,
                ins=[shard1.opt()], outs=[table1.opt()],
                replica_groups=[list(range(CORES))],
            )

            def layer(table, agg_tile, init_b, ps2):
                if init_b is not None:
                    bsrc = bass.AP(
                        tensor=init_b.tensor, offset=init_b[:].offset,
                        ap=[init_b[:].ap[0], [0, nwin], [1, 32]],
                    )
                    nc.vector.tensor_copy(
                        out=agg_tile[:].rearrange("p (v f) -> p v f", f=32), in_=bsrc
                    )
                else:
                    nc.vector.memset(agg_tile[:], 0.0)
                for kc0, gk, tier in ginstr:
                    nid = gk * 128          # idx per instruction
                    gx = gxp.tile([128, gk * 8], mybir.dt.int16, tag="gx")
                    nc.sync.dma_start(out=gx[:], in_=gidx[:, kc0 * 8 : (kc0 + gk) * 8])
                    w4 = gxp.tile([128, gk * 4], f32, tag="w4")
                    nc.sync.dma_start(out=w4[:], in_=w4g[:, kc0 * 4 : (kc0 + gk) * 4])
                    msgs = wp.tile([128, gk * 128], f32, tag="msgs")
                    emit_dma_gather(
                        nc.gpsimd,
                        out_ap=msgs[:].rearrange("p (k f) -> p k f", f=128),
                        in_ap=bass.AP(
                            tensor=table.tensor, offset=0,
                            ap=[[128, ngrp], [1, 128]],
                        ),
                        idxs_ap=gx[:],
                        num_idxs=nid,
                        elem_size=128,
                        elem_step=128,
                    )
                    # mask-weight multiply: [p, k*4, 32] *= w4 bcast over 32
                    nc.vector.tensor_tensor(
                        out=msgs[:].rearrange("p (q f) -> p q f", f=32),
                        in0=msgs[:].rearrange("p (q f) -> p q f", f=32),
                        in1=_b(w4[:], 32), op=mybir.AluOpType.mult,
                    )
                    def pairsum(dst_tile, src_tile, nblk):
                        # dst[i] = src[2i] + src[2i+1] over nblk 32-f blocks
                        nc.vector.tensor_tensor(
                            out=dst_tile[:].rearrange("p (q f) -> p q f", f=32),
                            in0=bass.AP(tensor=src_tile.tensor, offset=src_tile[:].offset,
                                        ap=[src_tile[:].ap[0], [64, nblk], [1, 32]]),
                            in1=bass.AP(tensor=src_tile.tensor, offset=src_tile[:].offset + 32,
                                        ap=[src_tile[:].ap[0], [64, nblk], [1, 32]]),
                            op=mybir.AluOpType.add,
                        )
                    s2 = wp.tile([128, gk * 64], f32, tag="s2")
                    pairsum(s2, msgs, gk * 2)
                    s1 = wp.tile([128, gk * 32], f32, tag="s1")
                    pairsum(s1, s2, gk)
                    # slot-sum: log2(tier) further pairsums
                    out1 = s1
                    nblk = gk // 2
                    d = 0
                    while (1 << d) < tier:
                        nxt = wp.tile([128, nblk * 32], f32, tag=f"tr{tier}_{d}")
                        pairsum(nxt, out1, nblk)
                        out1 = nxt
                        nblk //= 2
                        d += 1
                    # stage 2: per slot-column one-hot matmul into agg windows
                    for sc in range(gk // tier):
                        col = CB[tier] + (kc0 - KB[tier]) // tier + sc
                        cm = colmeta[col]
                        if cm is None:
                            continue
                        a_col, wins = cm
                        for wv in wins:
                            oh = sp.tile([128, 128], f32, tag="oh")
                            nc.vector.tensor_tensor(
                                out=oh[:],
                                in0=iota_t[:, (wv - a_col) * 128 : (wv - a_col + 1) * 128],
                                in1=_bcast_col(dloc_t[:, col : col + 1], 128),
                                op=mybir.AluOpType.is_equal,
                            )
                            pw = ps2.tile([128, 32], f32, tag="pw")
                            nc.tensor.matmul(
                                out=pw[:], lhsT=oh[:],
                                rhs=out1[:, sc * 32 : (sc + 1) * 32],
                                start=True, stop=True,
                            )
                            nc.vector.tensor_tensor(
                                out=agg_tile[:, wv * 32 : (wv + 1) * 32],
                                in0=agg_tile[:, wv * 32 : (wv + 1) * 32],
                                in1=pw[:], op=mybir.AluOpType.add,
                            )

            # ---- layer 1 ----
            agg1 = apool.tile([128, nwin * 32], f32, tag="agg1")
            with tc.tile_pool(name="ps2a", bufs=6, space="PSUM") as ps2:
                layer(table1, agg1, b1s, ps2)
            # relu -> shard2 (dense [nsh, 32]) -> AllGather -> table2
            h2cm = tc.tile_pool(name="h2p", bufs=1)
            h2pool = h2cm.__enter__()
            h2 = h2pool.tile([128, nwin * 32], f32, tag="h2")
            nc.scalar.activation(
                out=h2[:].rearrange("p (v f) -> p v f", f=32),
                in_=agg1[:].rearrange("p (v f) -> p v f", f=32), func=AF.Relu,
            )
            dst2 = bass.AP(tensor=shard2.tensor, offset=0,
                           ap=[[32, 128], [128 * 32, nwin], [1, 32]])
            nc.sync.dma_start(out=dst2, in_=h2[:].rearrange("p (v f) -> p v f", f=32))
            h2cm.__exit__(None, None, None)
            nc.gpsimd.collective_compute(
                "AllGather", mybir.AluOpType.bypass,
                ins=[shard2.opt()], outs=[table2.opt()],
                replica_groups=[list(range(CORES))],
            )

            # ---- layer 2 ----
            agg2 = apool.tile([128, nwin * 32], f32, tag="agg1")
            with tc.tile_pool(name="ps2b", bufs=6, space="PSUM") as ps2:
                layer(table2, agg2, None, ps2)

            # ---- out = log_softmax(agg2 @ W2 + b2) ----
            zall = apool.tile([128, nwin * F_OUT], f32, tag="zall")
            sall = apool.tile([128, nwin], f32, tag="sall")
            pf_cm = tc.tile_pool(name="pf", bufs=3, space="PSUM")
            pf = pf_cm.__enter__()
            for v in range(nwin):
                tp = pf.tile([F_HID, 128], f32, tag="tp")
                nc.tensor.transpose(
                    out=tp[:], in_=agg2[:, v * 32 : (v + 1) * 32], identity=id_s[:]
                )
                aT = sp.tile([F_HID, 128], f32, tag="aT")
                nc.vector.tensor_copy(out=aT[:], in_=tp[:])
                zp = pf.tile([128, F_OUT], f32, tag="zp")
                nc.tensor.matmul(out=zp[:], lhsT=aT[:], rhs=W2s[:], start=True, stop=False)
                nc.tensor.matmul(out=zp[:], lhsT=ones_s[:], rhs=b2s[:], start=False, stop=True)
                negm = sp.tile([128, 1], f32, tag="negm")
                nc.vector.reduce_max(out=negm[:], in_=zp[:], axis=AX, negate=True)
                nc.vector.tensor_tensor(
                    out=zall[:, v * F_OUT : (v + 1) * F_OUT],
                    in0=zp[:], in1=_bcast_col(negm[:], F_OUT),
                    op=mybir.AluOpType.add,
                )
                etmp = sp.tile([128, F_OUT], f32, tag="etmp")
                nc.scalar.activation(
                    out=etmp[:], in_=zall[:, v * F_OUT : (v + 1) * F_OUT],
                    func=AF.Exp, accum_out=sall[:, v : v + 1],
                )
            lns = apool.tile([128, nwin], f32, tag="lns")
            nc.scalar.activation(out=lns[:], in_=sall[:], func=AF.Ln)
            nc.vector.tensor_tensor(
                out=zall[:].rearrange("p (v f) -> p v f", f=F_OUT),
                in0=zall[:].rearrange("p (v f) -> p v f", f=F_OUT),
                in1=_b(lns[:], F_OUT),
                op=mybir.AluOpType.subtract,
            )
            outdst = bass.AP(
                tensor=out_t, offset=0,
                ap=[[F_OUT, 128], [128 * F_OUT, nwin], [1, F_OUT]],
            )
            nc.sync.dma_start(out=outdst, in_=zall[:].rearrange("p (v f) -> p v f", f=F_OUT))
            pf_cm.__exit__(None, None, None)

    nc.compile()
    return nc


def make_inmaps(meta, inmaps_edges, x, W1, b1, W2, b2):
    geo: Geo = meta["geo"]
    nsh = geo.nsh
    n = geo.n_nodes
    xT_full = np.zeros((F_IN, geo.ntab), np.float32)
    xT_full[:, :n] = np.asarray(x, np.float32).T
    iota = np.tile(np.arange(512, dtype=np.float32)[None, :], (128, 1))
    ident = np.eye(128, dtype=np.float32)
    b1b = np.tile(np.asarray(b1, np.float32)[None, :], (128, 1))
    consts = dict(
        iota512=iota, ident=ident,
        W1t=np.asarray(W1, np.float32), b1t=b1b,
        W2t=np.asarray(W2, np.float32), b2t=np.asarray(b2, np.float32)[None, :],
        onest=np.ones((1, 128), np.float32),
    )
    maps = []
    for c in range(CORES):
        m = dict(inmaps_edges[c])
        m.update(consts)
        m["xT"] = np.ascontiguousarray(xT_full[:, c * nsh : (c + 1) * nsh])
        maps.append(m)
    return maps


_CACHE = {}


def run(x, edge_index, edge_weight, W1, b1, W2, b2, geo=FULL, trace=False):
    meta, inmaps_edges = pack(edge_index, edge_weight, geo)
    key = ("geo%d" % geo.n_nodes, meta["S_T"])
    if key in _CACHE:
        nc = _CACHE[key]
    else:
        nc = build(meta)
        _CACHE[key] = nc
    maps = make_inmaps(meta, inmaps_edges, x, W1, b1, W2, b2)
    res = run_bass_kernel_spmd(nc, maps, core_ids=list(range(CORES)), trace=trace)
    n = geo.n_nodes
    out = np.empty((n, F_OUT), np.float32)
    for c in range(CORES):
        lo = c * geo.nsh
        hi = min(lo + geo.nsh, n)
        if hi > lo:
            out[lo:hi] = res.results[c]["out"][: hi - lo]
    return out, res


def kernel(x, edge_index, edge_weight, W1, b1, W2, b2):
    out, _ = run(
        np.asarray(x), np.asarray(edge_index), np.asarray(edge_weight),
        np.asarray(W1), np.asarray(b1), np.asarray(W2), np.asarray(b2),
    )
    return out


# revision 24
# speedup vs baseline: 2.4196x; 1.0135x over previous
"""GCN (2-layer, edge-weighted, log_softmax) on 8 Trainium2 NeuronCores.

Strategy v2 (dst-sharded edges, matmul-based segment-sum, 4-row gather):
  - Nodes sharded 12544/core. Feature tables stored dense node-major
    [100352, 32] f32, viewed as [25088, 128] (4 nodes per 512-B row).
  - Layer k: h = x @ Wk computed data-parallel on node shards -> AllGather
    into the replicated dense table in HBM.
  - Per-edge gather via InstDMAGatherAnt with idx = src//4 (int16 fits
    without group-splitting the table), elem 512 B = the 4-node row; the
    1-of-4 row select folds into the weight multiply (w4 mask grid).
  - Edges packed into 8-edge same-destination slots; DVE applies the w4
    mask-mult and a 4->1 + 8->1 tree reduction; a per-column one-hot
    (is_equal vs iota) matmul segment-sums slot partials into PSUM windows
    of 128 destinations, accumulated into SBUF agg [d%128, (d//128)*32+f].
  - 2048-idx gather instructions keep the SWDGE generation (8 ns/idx, the
    bottleneck) back-to-back while descriptor drains pipeline behind it.
  - Layer 2 aggregates relu(agg1 + b1) with the identical edge structure,
    then applies W2 (+b2) per 128-node window and an on-chip log_softmax.
Host side only packs indices/weights/slot metadata (numpy).
"""

import os
import sys

for _p in ("/opt/trn_rl_repo", "/root/.axon_site/_ro/trn_rl_repo"):
    if os.path.isdir(_p) and _p not in sys.path:
        sys.path.insert(0, _p)

import numpy as np

import concourse.ap_utils as ap_utils
import concourse.bass as bass
import concourse.mybir as mybir
from concourse import bacc, tile
from concourse.bass_utils import run_bass_kernel_spmd

CORES = 8
F_IN = 128
F_HID = 32
F_OUT = 40
KSLOT = 8       # edges per slot (same destination)
GK = 16         # idx-columns per gather instruction (16*128 = 2048 idx)


class Geo:
    def __init__(self, n_nodes=100000, nsh=12544):
        self.n_nodes = n_nodes
        self.nsh = nsh                    # nodes per core shard (mult of 512)
        self.ntab = nsh * CORES           # table nodes (100352)
        self.ngrp = self.ntab // 4        # 4-node gather rows (25088)
        assert nsh % 128 == 0 and nsh % 4 == 0
        self.nwin = nsh // 128            # 128-destination windows per core


FULL = Geo()


def _wrap16(flat, T):
    """token i -> [i%16, i//16], replicated to 128 partitions."""
    a = flat.reshape(T // 16, 16).T
    return np.tile(a, (8, 1)).copy()


TIERS = (8, 4, 2, 1)


def pack(edge_index, edge_weight, geo: Geo):
    """Group edges by (core, dst) into same-destination slots of size 8/4/2/1
    (binary decomposition of each run length); build the shared
    column->window template per tier and all per-core device arrays."""
    src = np.asarray(edge_index[0], dtype=np.int64)
    dst = np.asarray(edge_index[1], dtype=np.int64)
    w = np.asarray(edge_weight, dtype=np.float32)
    nsh, nwin = geo.nsh, geo.nwin

    core = dst // nsh
    pc = []
    cnt = {t: np.zeros((CORES, nwin), np.int64) for t in TIERS}
    for c in range(CORES):
        m = core == c
        order = np.argsort(dst[m], kind="stable")
        sk = src[m][order]
        dlk = (dst[m] - c * nsh)[order]
        wk = w[m][order]
        new = np.r_[True, dlk[1:] != dlk[:-1]]
        run_first = np.flatnonzero(new)
        run_len = np.diff(np.r_[run_first, len(dlk)])
        run_id = np.cumsum(new) - 1
        rank = np.arange(len(dlk)) - run_first[run_id]
        dl_run = dlk[run_first]
        v_run = dl_run // 128
        # per-run slot counts per tier (binary decomposition)
        n_t = {8: run_len // 8, 4: (run_len % 8) // 4,
               2: (run_len % 4) // 2, 1: run_len % 2}
        for t in TIERS:
            np.add.at(cnt[t][c], v_run, n_t[t])
        pc.append((sk, dlk, wk, rank, run_id, run_len, dl_run, v_run, n_t))

    # shared template per tier: window capacity = max over cores, round to 8;
    # section slot count S_t padded so S_t * t is a multiple of 2048 tokens.
    off, S, CB, KB, TB = {}, {}, {}, {}, {}
    cols_acc = 0
    kcol_acc = 0
    tok_acc = 0
    colmeta = []
    for t in TIERS:
        cap = cnt[t].max(axis=0)
        cap = ((cap + 7) // 8) * 8
        o = np.zeros(nwin, np.int64)
        b = 0
        for v in range(nwin):
            o[v] = b
            b += cap[v]
        align = 2048 // t
        S_t = int((b + align - 1) // align * align)
        off[t], S[t] = o, S_t
        CB[t], KB[t], TB[t] = cols_acc, kcol_acc, tok_acc
        flat_off = np.r_[o, S_t]
        for col in range(S_t // 128):
            lo, hi = col * 128, col * 128 + 128
            i0 = int(np.searchsorted(flat_off, lo, side="right") - 1)
            wins = []
            for v in range(max(i0, 0), nwin):
                if flat_off[v] >= hi:
                    break
                if flat_off[v + 1] <= lo:
                    continue
                wins.append(v)
            if wins:
                assert wins[-1] - wins[0] < 8, "column spans too many windows"
            colmeta.append((wins[0], wins) if wins else None)
        cols_acc += S_t // 128
        kcol_acc += S_t * t // 128
        tok_acc += S_t * t
    COLS, KC, T = cols_acc, kcol_acc, tok_acc

    ginstr = []  # (kc0 global, gk, tier)
    for t in TIERS:
        kc0, kc1 = KB[t], KB[t] + S[t] * t // 128
        kc = kc0
        while kc < kc1:
            ginstr.append((kc, min(GK, kc1 - kc), t))
            kc += GK

    # per-core arrays
    inmaps = []
    for c in range(CORES):
        sk, dlk, wk, rank, run_id, run_len, dl_run, v_run, n_t = pc[c]
        idx_flat = np.zeros(T, np.int16)
        w4_flat = np.zeros((T, 4), np.float32)
        dl_slot_all = np.full(COLS * 128, float(geo.n_nodes), np.float32)
        # rank boundaries within each run for tier assignment
        l8 = run_len // 8 * 8
        lo_t = {8: np.zeros(len(run_len), np.int64), 4: l8,
                2: l8 + n_t[4] * 4, 1: l8 + n_t[4] * 4 + n_t[2] * 2}
        for t in TIERS:
            nsl = n_t[t]
            csum = np.cumsum(nsl)
            start_excl = np.r_[0, csum[:-1]]
            newv = np.r_[True, v_run[1:] != v_run[:-1]]
            v_first = np.flatnonzero(newv)
            v_id = np.cumsum(newv) - 1
            base_in_v = start_excl - start_excl[v_first][v_id]
            run_slot = off[t][v_run] + base_in_v  # slot within tier section
            # edges of this tier: rank in [lo_t[t][run], lo_t[t][run]+nsl*t)
            rr = rank - lo_t[t][run_id]
            sel = (rr >= 0) & (rr < nsl[run_id] * t)
            rsel = rr[sel]
            rid = run_id[sel]
            slot_e = run_slot[rid] + rsel // t
            j_e = rsel % t
            tok = TB[t] + (slot_e // 128 * t + j_e) * 128 + slot_e % 128
            idx_flat[tok] = (sk[sel] // 4).astype(np.int16)
            w4_flat[tok, sk[sel] % 4] = wk[sel]
            # slot dst values for this tier's columns
            n_runs = len(nsl)
            reps = np.repeat(np.arange(n_runs), nsl)
            ar = np.arange(len(reps)) - np.repeat(start_excl, nsl)
            pos = np.repeat(run_slot, nsl) + ar
            dl_slot_all[CB[t] * 128 + pos] = np.repeat(dl_run, nsl).astype(np.float32)
        dcol = dl_slot_all.reshape(COLS, 128).T.copy()
        for col in range(COLS):
            if colmeta[col] is not None:
                dcol[:, col] -= 128.0 * colmeta[col][0]

        inmaps.append(
            dict(
                gidx=_wrap16(idx_flat, T),
                w4g=w4_flat.reshape(KC, 128, 4).transpose(1, 0, 2).reshape(128, KC * 4).copy(),
                dloc=dcol,
            )
        )

    meta = dict(S_T=COLS * 128, COLS=COLS, T=T, KC=KC, colmeta=colmeta,
                ginstr=ginstr, CB=CB, KB=KB, geo=geo)
    return meta, inmaps


def emit_dma_gather(gp, out_ap, in_ap, idxs_ap, num_idxs, elem_size, elem_step,
                    single_packet=False):
    """bass.dma_gather minus the blanket 256B elem assert."""
    from concourse.bass import exact_div

    assert idxs_ap.dtype == mybir.dt.int16
    assert in_ap.dtype == out_ap.dtype
    assert in_ap.space == bass.MemorySpace.DRAM
    assert ap_utils.ap_is_contiguous(in_ap.ap[1:])
    assert ap_utils.ap_is_contiguous(out_ap.ap[1:])
    assert ap_utils.ap_is_contiguous(idxs_ap.ap[1:])
    assert in_ap.ap[-1][1] == out_ap.ap[-1][1] == elem_size
    assert out_ap.ap[0][1] * out_ap.ap[1][1] == num_idxs
    assert in_ap.ap[0][0] == elem_step
    stride_bytes_256 = exact_div(elem_step * mybir.dt.size(in_ap.dtype), 256)
    assert stride_bytes_256 < 256
    _in_ap = gp.lower_ap_dma(in_ap, for_custom_bir_dma=True)
    _idxs_ap = gp.lower_ap(idxs_ap)
    _out_ap = gp.lower_ap(out_ap)
    return gp.add_instruction(
        mybir.InstDMAGatherAnt(
            name=gp.bass.get_next_instruction_name(),
            ins=[*_in_ap, _idxs_ap, gp.lower_val_access(gp.to_reg(num_idxs))],
            outs=[_out_ap],
            transpose=False,
            num_idxs=num_idxs,
            elem_size=elem_size,
            stride_bytes_256=stride_bytes_256,
            gen_mode=0,
            single_packet=single_packet,
            queue_num=0,
            sbuf_tokens_per_rank=0,
            sbuf_free_dim_per_rank=0,
            sbuf_free_dim_pad_per_rank=0,
            sbuf_byte_offset=0,
        )
    )


def _b(ap2, reps):
    """broadcast each element of an AP over `reps` trailing copies."""
    return bass.AP(tensor=ap2.tensor, offset=ap2.offset, ap=[*ap2.ap, [0, reps]])


def _bcast_col(ap1, n):
    """[P, 1] AP -> [P, n] zero-stride broadcast."""
    return bass.AP(tensor=ap1.tensor, offset=ap1.offset, ap=[ap1.ap[0], [0, n]])


def build(meta):
    geo: Geo = meta["geo"]
    S_T, COLS, T, KC = meta["S_T"], meta["COLS"], meta["T"], meta["KC"]
    colmeta, ginstr = meta["colmeta"], meta["ginstr"]
    CB, KB = meta["CB"], meta["KB"]
    nsh, ngrp, nwin = geo.nsh, geo.ngrp, geo.nwin
    f32 = mybir.dt.float32
    AX = mybir.AxisListType.X
    AF = mybir.ActivationFunctionType

    nc = bacc.Bacc("TRN2", target_bir_lowering=False, debug=False, num_devices=CORES)

    xT = nc.dram_tensor("xT", [F_IN, nsh], f32, kind="ExternalInput")
    gidx = nc.dram_tensor("gidx", [128, T // 16], mybir.dt.int16, kind="ExternalInput")
    w4g = nc.dram_tensor("w4g", [128, KC * 4], f32, kind="ExternalInput")
    dloc = nc.dram_tensor("dloc", [128, COLS], f32, kind="ExternalInput")
    iota512 = nc.dram_tensor("iota512", [128, 1024], f32, kind="ExternalInput")
    ident = nc.dram_tensor("ident", [128, 128], f32, kind="ExternalInput")
    W1t = nc.dram_tensor("W1t", [F_IN, F_HID], f32, kind="ExternalInput")
    b1t = nc.dram_tensor("b1t", [128, F_HID], f32, kind="ExternalInput")
    W2t = nc.dram_tensor("W2t", [128, F_OUT], f32, kind="ExternalInput")
    b2t = nc.dram_tensor("b2t", [1, F_OUT], f32, kind="ExternalInput")
    onest = nc.dram_tensor("onest", [1, 128], f32, kind="ExternalInput")
    out_t = nc.dram_tensor("out", [nsh, F_OUT], f32, kind="ExternalOutput")

    with tile.TileContext(nc) as tc:
        with (
            tc.tile_pool(name="const", bufs=1) as cpool,
            tc.tile_pool(name="dram", bufs=1, space="DRAM") as dram,
            tc.tile_pool(name="gxp", bufs=8) as gxp,
            tc.tile_pool(name="work", bufs=4) as wp,
            tc.tile_pool(name="scol", bufs=8) as sp,
            tc.tile_pool(name="agg", bufs=1) as apool,
        ):
            iota_t = cpool.tile([128, 1024], f32)
            nc.sync.dma_start(out=iota_t[:], in_=iota512[:, :])
            dloc_t = cpool.tile([128, COLS], f32)
            nc.sync.dma_start(out=dloc_t[:], in_=dloc[:, :])
            W1s = cpool.tile([F_IN, F_HID], f32)
            nc.sync.dma_start(out=W1s[:], in_=W1t[:, :])
            b1s = cpool.tile([128, F_HID], f32)
            nc.sync.dma_start(out=b1s[:], in_=b1t[:, :])
            W2s = cpool.tile([128, F_OUT], f32)
            nc.sync.dma_start(out=W2s[:], in_=W2t[:, :])
            b2s = cpool.tile([1, F_OUT], f32)
            nc.sync.dma_start(out=b2s[:], in_=b2t[:, :])
            ones_s = cpool.tile([1, 128], f32)
            nc.sync.dma_start(out=ones_s[:], in_=onest[:, :])
            id_s = cpool.tile([128, 128], f32)
            nc.sync.dma_start(out=id_s[:], in_=ident[:, :])

            shard1 = dram.tile([nsh // 4, 128], f32)
            shard2 = dram.tile([nsh // 4, 128], f32)
            table1 = dram.tile([ngrp, 128], f32, addr_space="Shared")
            table2 = dram.tile([ngrp, 128], f32, addr_space="Shared")

            # ---- h = x @ W1 on own shard -> shard1 (dense [nsh, 32]) ----
            with (
                tc.tile_pool(name="xt", bufs=2) as xp,
                tc.tile_pool(name="ph", bufs=4, space="PSUM") as ph,
            ):
                ntiles = nsh // 128
                quarter = ((ntiles + 3) // 4) * 128
                for hh in range(4):
                    hi = min(quarter * (hh + 1), nsh)
                    lo = quarter * hh
                    if hi <= lo:
                        continue
                    xTs = xp.tile([F_IN, quarter], f32, tag="xts")
                    nc.sync.dma_start(out=xTs[:, : hi - lo], in_=xT[:, lo:hi])
                    tt = 0
                    while tt * 128 < hi - lo:
                        # batch 4 matmul outputs into one 16KB-contig write
                        nb = min(4, (hi - lo) // 128 - tt)
                        hs = wp.tile([128, 4 * F_HID], f32, tag="hsb")
                        for q in range(nb):
                            hp = ph.tile([128, F_HID], f32, tag="hps")
                            nc.tensor.matmul(
                                out=hp[:], lhsT=xTs[:, (tt + q) * 128 : (tt + q + 1) * 128],
                                rhs=W1s[:], start=True, stop=True,
                            )
                            nc.vector.tensor_copy(
                                out=hs[:, q * F_HID : (q + 1) * F_HID], in_=hp[:]
                            )
                        t = (lo // 128) + tt
                        dst = bass.AP(
                            tensor=shard1.tensor, offset=t * 128 * 32,
                            ap=[[32, 128], [128 * 32, nb], [1, 32]],
                        )
                        nc.sync.dma_start(
                            out=dst,
                            in_=hs[:, : nb * F_HID].rearrange("p (b f) -> p b f", f=32),
                        )
                        tt += nb

            nc.gpsimd.collective_compute(
                "AllGather", mybir.AluOpType.bypass,
                ins=[shard1.opt()], outs=[table1.opt()],
                replica_groups=[list(range(CORES))],
            )

            def layer(table, agg_tile, init_b, ps2):
                if init_b is not None:
                    bsrc = bass.AP(
                        tensor=init_b.tensor, offset=init_b[:].offset,
                        ap=[init_b[:].ap[0], [0, nwin], [1, 32]],
                    )
                    nc.vector.tensor_copy(
                        out=agg_tile[:].rearrange("p (v f) -> p v f", f=32), in_=bsrc
                    )
                else:
                    nc.vector.memset(agg_tile[:], 0.0)
                for kc0, gk, tier in ginstr:
                    nid = gk * 128          # idx per instruction
                    gx = gxp.tile([128, gk * 8], mybir.dt.int16, tag="gx")
                    nc.sync.dma_start(out=gx[:], in_=gidx[:, kc0 * 8 : (kc0 + gk) * 8])
                    w4 = gxp.tile([128, gk * 4], f32, tag="w4")
                    nc.sync.dma_start(out=w4[:], in_=w4g[:, kc0 * 4 : (kc0 + gk) * 4])
                    msgs = wp.tile([128, gk * 128], f32, tag="msgs")
                    emit_dma_gather(
                        nc.gpsimd,
                        out_ap=msgs[:].rearrange("p (k f) -> p k f", f=128),
                        in_ap=bass.AP(
                            tensor=table.tensor, offset=0,
                            ap=[[128, ngrp], [1, 128]],
                        ),
                        idxs_ap=gx[:],
                        num_idxs=nid,
                        elem_size=128,
                        elem_step=128,
                    )
                    # mask-weight multiply: [p, k*4, 32] *= w4 bcast over 32
                    nc.vector.tensor_tensor(
                        out=msgs[:].rearrange("p (q f) -> p q f", f=32),
                        in0=msgs[:].rearrange("p (q f) -> p q f", f=32),
                        in1=_b(w4[:], 32), op=mybir.AluOpType.mult,
                    )
                    def pairsum(dst_tile, src_tile, nblk):
                        # dst[i] = src[2i] + src[2i+1] over nblk 32-f blocks
                        nc.vector.tensor_tensor(
                            out=dst_tile[:].rearrange("p (q f) -> p q f", f=32),
                            in0=bass.AP(tensor=src_tile.tensor, offset=src_tile[:].offset,
                                        ap=[src_tile[:].ap[0], [64, nblk], [1, 32]]),
                            in1=bass.AP(tensor=src_tile.tensor, offset=src_tile[:].offset + 32,
                                        ap=[src_tile[:].ap[0], [64, nblk], [1, 32]]),
                            op=mybir.AluOpType.add,
                        )
                    s2 = wp.tile([128, gk * 64], f32, tag="s2")
                    pairsum(s2, msgs, gk * 2)
                    s1 = wp.tile([128, gk * 32], f32, tag="s1")
                    pairsum(s1, s2, gk)
                    # slot-sum: log2(tier) further pairsums
                    out1 = s1
                    nblk = gk // 2
                    d = 0
                    while (1 << d) < tier:
                        nxt = wp.tile([128, nblk * 32], f32, tag=f"tr{tier}_{d}")
                        pairsum(nxt, out1, nblk)
                        out1 = nxt
                        nblk //= 2
                        d += 1
                    # stage 2: per slot-column one-hot matmul into agg windows
                    for sc in range(gk // tier):
                        col = CB[tier] + (kc0 - KB[tier]) // tier + sc
                        cm = colmeta[col]
                        if cm is None:
                            continue
                        a_col, wins = cm
                        for wv in wins:
                            oh = sp.tile([128, 128], f32, tag="oh")
                            nc.vector.tensor_tensor(
                                out=oh[:],
                                in0=iota_t[:, (wv - a_col) * 128 : (wv - a_col + 1) * 128],
                                in1=_bcast_col(dloc_t[:, col : col + 1], 128),
                                op=mybir.AluOpType.is_equal,
                            )
                            pw = ps2.tile([128, 32], f32, tag="pw")
                            nc.tensor.matmul(
                                out=pw[:], lhsT=oh[:],
                                rhs=out1[:, sc * 32 : (sc + 1) * 32],
                                start=True, stop=True,
                            )
                            nc.vector.tensor_tensor(
                                out=agg_tile[:, wv * 32 : (wv + 1) * 32],
                                in0=agg_tile[:, wv * 32 : (wv + 1) * 32],
                                in1=pw[:], op=mybir.AluOpType.add,
                            )

            # ---- layer 1 ----
            agg1 = apool.tile([128, nwin * 32], f32, tag="agg1")
            with tc.tile_pool(name="ps2a", bufs=6, space="PSUM") as ps2:
                layer(table1, agg1, b1s, ps2)
            # relu -> shard2 (dense [nsh, 32]) -> AllGather -> table2
            h2cm = tc.tile_pool(name="h2p", bufs=1)
            h2pool = h2cm.__enter__()
            h2 = h2pool.tile([128, nwin * 32], f32, tag="h2")
            nc.scalar.activation(
                out=h2[:].rearrange("p (v f) -> p v f", f=32),
                in_=agg1[:].rearrange("p (v f) -> p v f", f=32), func=AF.Relu,
            )
            dst2 = bass.AP(tensor=shard2.tensor, offset=0,
                           ap=[[32, 128], [128 * 32, nwin], [1, 32]])
            nc.sync.dma_start(out=dst2, in_=h2[:].rearrange("p (v f) -> p v f", f=32))
            h2cm.__exit__(None, None, None)
            nc.gpsimd.collective_compute(
                "AllGather", mybir.AluOpType.bypass,
                ins=[shard2.opt()], outs=[table2.opt()],
                replica_groups=[list(range(CORES))],
            )

            # ---- layer 2 ----
            agg2 = apool.tile([128, nwin * 32], f32, tag="agg1")
            with tc.tile_pool(name="ps2b", bufs=6, space="PSUM") as ps2:
                layer(table2, agg2, None, ps2)

            # ---- out = log_softmax(agg2 @ W2 + b2) ----
            zall = apool.tile([128, nwin * F_OUT], f32, tag="zall")
            sall = apool.tile([128, nwin], f32, tag="sall")
            pf_cm = tc.tile_pool(name="pf", bufs=3, space="PSUM")
            pf = pf_cm.__enter__()
            for v in range(nwin):
                tp = pf.tile([F_HID, 128], f32, tag="tp")
                nc.tensor.transpose(
                    out=tp[:], in_=agg2[:, v * 32 : (v + 1) * 32], identity=id_s[:]
                )
                aT = sp.tile([F_HID, 128], f32, tag="aT")
                nc.vector.tensor_copy(out=aT[:], in_=tp[:])
                zp = pf.tile([128, F_OUT], f32, tag="zp")
                nc.tensor.matmul(out=zp[:], lhsT=aT[:], rhs=W2s[:32, :], start=True, stop=False)
                nc.tensor.matmul(out=zp[:], lhsT=ones_s[:], rhs=b2s[:], start=False, stop=True)
                negm = sp.tile([128, 1], f32, tag="negm")
                nc.vector.reduce_max(out=negm[:], in_=zp[:], axis=AX, negate=True)
                nc.vector.tensor_tensor(
                    out=zall[:, v * F_OUT : (v + 1) * F_OUT],
                    in0=zp[:], in1=_bcast_col(negm[:], F_OUT),
                    op=mybir.AluOpType.add,
                )
                etmp = sp.tile([128, F_OUT], f32, tag="etmp")
                nc.scalar.activation(
                    out=etmp[:], in_=zall[:, v * F_OUT : (v + 1) * F_OUT],
                    func=AF.Exp, accum_out=sall[:, v : v + 1],
                )
            lns = apool.tile([128, nwin], f32, tag="lns")
            nc.scalar.activation(out=lns[:], in_=sall[:], func=AF.Ln)
            nc.vector.tensor_tensor(
                out=zall[:].rearrange("p (v f) -> p v f", f=F_OUT),
                in0=zall[:].rearrange("p (v f) -> p v f", f=F_OUT),
                in1=_b(lns[:], F_OUT),
                op=mybir.AluOpType.subtract,
            )
            outdst = bass.AP(
                tensor=out_t, offset=0,
                ap=[[F_OUT, 128], [128 * F_OUT, nwin], [1, F_OUT]],
            )
            nc.sync.dma_start(out=outdst, in_=zall[:].rearrange("p (v f) -> p v f", f=F_OUT))
            pf_cm.__exit__(None, None, None)

    nc.compile()
    return nc


def make_inmaps(meta, inmaps_edges, x, W1, b1, W2, b2):
    geo: Geo = meta["geo"]
    nsh = geo.nsh
    n = geo.n_nodes
    xT_full = np.zeros((F_IN, geo.ntab), np.float32)
    xT_full[:, :n] = np.asarray(x, np.float32).T
    iota = np.tile(np.arange(1024, dtype=np.float32)[None, :], (128, 1))
    ident = np.eye(128, dtype=np.float32)
    b1b = np.tile(np.asarray(b1, np.float32)[None, :], (128, 1))
    consts = dict(
        iota512=iota, ident=ident,
        W1t=np.asarray(W1, np.float32), b1t=b1b,
        W2t=np.tile(np.asarray(W2, np.float32), (4, 1)),
        b2t=np.asarray(b2, np.float32)[None, :],
        onest=np.ones((1, 128), np.float32),
    )
    maps = []
    for c in range(CORES):
        m = dict(inmaps_edges[c])
        m.update(consts)
        m["xT"] = np.ascontiguousarray(xT_full[:, c * nsh : (c + 1) * nsh])
        maps.append(m)
    return maps


_CACHE = {}


def run(x, edge_index, edge_weight, W1, b1, W2, b2, geo=FULL, trace=False):
    meta, inmaps_edges = pack(edge_index, edge_weight, geo)
    key = ("geo%d" % geo.n_nodes, meta["S_T"])
    if key in _CACHE:
        nc = _CACHE[key]
    else:
        nc = build(meta)
        _CACHE[key] = nc
    maps = make_inmaps(meta, inmaps_edges, x, W1, b1, W2, b2)
    res = run_bass_kernel_spmd(nc, maps, core_ids=list(range(CORES)), trace=trace)
    n = geo.n_nodes
    out = np.empty((n, F_OUT), np.float32)
    for c in range(CORES):
        lo = c * geo.nsh
        hi = min(lo + geo.nsh, n)
        if hi > lo:
            out[lo:hi] = res.results[c]["out"][: hi - lo]
    return out, res


def kernel(x, edge_index, edge_weight, W1, b1, W2, b2):
    out, _ = run(
        np.asarray(x), np.asarray(edge_index), np.asarray(edge_weight),
        np.asarray(W1), np.asarray(b1), np.asarray(W2), np.asarray(b2),
    )
    return out


# revision 32
# speedup vs baseline: 2.4918x; 1.0299x over previous
"""GCN (2-layer, edge-weighted, log_softmax) on 8 Trainium2 NeuronCores.

Strategy v2 (dst-sharded edges, matmul-based segment-sum, 4-row gather):
  - Nodes sharded 12544/core. Feature tables stored dense node-major
    [100352, 32] f32, viewed as [25088, 128] (4 nodes per 512-B row).
  - Layer k: h = x @ Wk computed data-parallel on node shards -> AllGather
    into the replicated dense table in HBM.
  - Per-edge gather via InstDMAGatherAnt with idx = src//4 (int16 fits
    without group-splitting the table), elem 512 B = the 4-node row; the
    1-of-4 row select folds into the weight multiply (w4 mask grid).
  - Edges packed into 8-edge same-destination slots; DVE applies the w4
    mask-mult and a 4->1 + 8->1 tree reduction; a per-column one-hot
    (is_equal vs iota) matmul segment-sums slot partials into PSUM windows
    of 128 destinations, accumulated into SBUF agg [d%128, (d//128)*32+f].
  - 2048-idx gather instructions keep the SWDGE generation (8 ns/idx, the
    bottleneck) back-to-back while descriptor drains pipeline behind it.
  - Layer 2 aggregates relu(agg1 + b1) with the identical edge structure,
    then applies W2 (+b2) per 128-node window and an on-chip log_softmax.
Host side only packs indices/weights/slot metadata (numpy).
"""

import os
import sys

for _p in ("/opt/trn_rl_repo", "/root/.axon_site/_ro/trn_rl_repo"):
    if os.path.isdir(_p) and _p not in sys.path:
        sys.path.insert(0, _p)

import numpy as np

import concourse.ap_utils as ap_utils
import concourse.bass as bass
import concourse.mybir as mybir
from concourse import bacc, tile
from concourse.bass_utils import run_bass_kernel_spmd

CORES = 8
F_IN = 128
F_HID = 32
F_OUT = 40
KSLOT = 8       # edges per slot (same destination)
GK = 16         # idx-columns per gather instruction (16*128 = 2048 idx)


class Geo:
    def __init__(self, n_nodes=100000, nsh=12544):
        self.n_nodes = n_nodes
        self.nsh = nsh                    # nodes per core shard (mult of 512)
        self.ntab = nsh * CORES           # table nodes (100352)
        self.ngrp = self.ntab // 4        # 4-node gather rows (25088)
        assert nsh % 128 == 0 and nsh % 4 == 0
        self.nwin = nsh // 128            # 128-destination windows per core


FULL = Geo()


def _wrap16(flat, T):
    """token i -> [i%16, i//16], replicated to 128 partitions."""
    a = flat.reshape(T // 16, 16).T
    return np.tile(a, (8, 1)).copy()


TIERS = (8, 4, 2, 1)


def pack(edge_index, edge_weight, geo: Geo):
    """Group edges by (core, dst) into same-destination slots of size 8/4/2/1
    (binary decomposition of each run length); build the shared
    column->window template per tier and all per-core device arrays."""
    src = np.asarray(edge_index[0], dtype=np.int64)
    dst = np.asarray(edge_index[1], dtype=np.int64)
    w = np.asarray(edge_weight, dtype=np.float32)
    nsh, nwin = geo.nsh, geo.nwin

    core = dst // nsh
    pc = []
    cnt = {t: np.zeros((CORES, nwin), np.int64) for t in TIERS}
    for c in range(CORES):
        m = core == c
        order = np.argsort(dst[m], kind="stable")
        sk = src[m][order]
        dlk = (dst[m] - c * nsh)[order]
        wk = w[m][order]
        new = np.r_[True, dlk[1:] != dlk[:-1]]
        run_first = np.flatnonzero(new)
        run_len = np.diff(np.r_[run_first, len(dlk)])
        run_id = np.cumsum(new) - 1
        rank = np.arange(len(dlk)) - run_first[run_id]
        dl_run = dlk[run_first]
        v_run = dl_run // 128
        # per-run slot counts per tier (binary decomposition)
        n_t = {8: run_len // 8, 4: (run_len % 8) // 4,
               2: (run_len % 4) // 2, 1: run_len % 2}
        for t in TIERS:
            np.add.at(cnt[t][c], v_run, n_t[t])
        pc.append((sk, dlk, wk, rank, run_id, run_len, dl_run, v_run, n_t))

    # shared template per tier: window capacity = max over cores, round to 8;
    # section slot count S_t padded so S_t * t is a multiple of 2048 tokens.
    off, S, CB, KB, TB = {}, {}, {}, {}, {}
    cols_acc = 0
    kcol_acc = 0
    tok_acc = 0
    colmeta = []
    for t in TIERS:
        cap = cnt[t].max(axis=0)
        o = np.zeros(nwin, np.int64)
        b = 0
        for v in range(nwin):
            o[v] = b
            b += cap[v]
        align = 2048 // t
        S_t = int((b + align - 1) // align * align)
        off[t], S[t] = o, S_t
        CB[t], KB[t], TB[t] = cols_acc, kcol_acc, tok_acc
        flat_off = np.r_[o, S_t]
        for col in range(S_t // 128):
            lo, hi = col * 128, col * 128 + 128
            i0 = int(np.searchsorted(flat_off, lo, side="right") - 1)
            wins = []
            for v in range(max(i0, 0), nwin):
                if flat_off[v] >= hi:
                    break
                if flat_off[v + 1] <= lo:
                    continue
                wins.append(v)
            if wins:
                assert wins[-1] - wins[0] < 8, "column spans too many windows"
            colmeta.append((wins[0], wins) if wins else None)
        cols_acc += S_t // 128
        kcol_acc += S_t * t // 128
        tok_acc += S_t * t
    COLS, KC, T = cols_acc, kcol_acc, tok_acc

    ginstr = []  # (kc0 global, gk, tier)
    for t in TIERS:
        kc0, kc1 = KB[t], KB[t] + S[t] * t // 128
        kc = kc0
        while kc < kc1:
            ginstr.append((kc, min(GK, kc1 - kc), t))
            kc += GK

    # per-core arrays
    inmaps = []
    for c in range(CORES):
        sk, dlk, wk, rank, run_id, run_len, dl_run, v_run, n_t = pc[c]
        idx_flat = np.zeros(T, np.int16)
        w4_flat = np.zeros((T, 4), np.float32)
        dl_slot_all = np.full(COLS * 128, float(geo.n_nodes), np.float32)
        # rank boundaries within each run for tier assignment
        l8 = run_len // 8 * 8
        lo_t = {8: np.zeros(len(run_len), np.int64), 4: l8,
                2: l8 + n_t[4] * 4, 1: l8 + n_t[4] * 4 + n_t[2] * 2}
        for t in TIERS:
            nsl = n_t[t]
            csum = np.cumsum(nsl)
            start_excl = np.r_[0, csum[:-1]]
            newv = np.r_[True, v_run[1:] != v_run[:-1]]
            v_first = np.flatnonzero(newv)
            v_id = np.cumsum(newv) - 1
            base_in_v = start_excl - start_excl[v_first][v_id]
            run_slot = off[t][v_run] + base_in_v  # slot within tier section
            # edges of this tier: rank in [lo_t[t][run], lo_t[t][run]+nsl*t)
            rr = rank - lo_t[t][run_id]
            sel = (rr >= 0) & (rr < nsl[run_id] * t)
            rsel = rr[sel]
            rid = run_id[sel]
            slot_e = run_slot[rid] + rsel // t
            j_e = rsel % t
            tok = TB[t] + (slot_e // 128 * t + j_e) * 128 + slot_e % 128
            idx_flat[tok] = (sk[sel] // 4).astype(np.int16)
            w4_flat[tok, sk[sel] % 4] = wk[sel]
            # slot dst values for this tier's columns
            n_runs = len(nsl)
            reps = np.repeat(np.arange(n_runs), nsl)
            ar = np.arange(len(reps)) - np.repeat(start_excl, nsl)
            pos = np.repeat(run_slot, nsl) + ar
            dl_slot_all[CB[t] * 128 + pos] = np.repeat(dl_run, nsl).astype(np.float32)
        dcol = dl_slot_all.reshape(COLS, 128).T.copy()
        for col in range(COLS):
            if colmeta[col] is not None:
                dcol[:, col] -= 128.0 * colmeta[col][0]

        inmaps.append(
            dict(
                gidx=_wrap16(idx_flat, T),
                w4g=w4_flat.reshape(KC, 128, 4).transpose(1, 0, 2).reshape(128, KC * 4).copy(),
                dloc=dcol,
            )
        )

    meta = dict(S_T=COLS * 128, COLS=COLS, T=T, KC=KC, colmeta=colmeta,
                ginstr=ginstr, CB=CB, KB=KB, geo=geo)
    return meta, inmaps


def emit_dma_gather(gp, out_ap, in_ap, idxs_ap, num_idxs, elem_size, elem_step,
                    single_packet=False):
    """bass.dma_gather minus the blanket 256B elem assert."""
    from concourse.bass import exact_div

    assert idxs_ap.dtype == mybir.dt.int16
    assert in_ap.dtype == out_ap.dtype
    assert in_ap.space == bass.MemorySpace.DRAM
    assert ap_utils.ap_is_contiguous(in_ap.ap[1:])
    assert ap_utils.ap_is_contiguous(out_ap.ap[1:])
    assert ap_utils.ap_is_contiguous(idxs_ap.ap[1:])
    assert in_ap.ap[-1][1] == out_ap.ap[-1][1] == elem_size
    assert out_ap.ap[0][1] * out_ap.ap[1][1] == num_idxs
    assert in_ap.ap[0][0] == elem_step
    stride_bytes_256 = exact_div(elem_step * mybir.dt.size(in_ap.dtype), 256)
    assert stride_bytes_256 < 256
    _in_ap = gp.lower_ap_dma(in_ap, for_custom_bir_dma=True)
    _idxs_ap = gp.lower_ap(idxs_ap)
    _out_ap = gp.lower_ap(out_ap)
    return gp.add_instruction(
        mybir.InstDMAGatherAnt(
            name=gp.bass.get_next_instruction_name(),
            ins=[*_in_ap, _idxs_ap, gp.lower_val_access(gp.to_reg(num_idxs))],
            outs=[_out_ap],
            transpose=False,
            num_idxs=num_idxs,
            elem_size=elem_size,
            stride_bytes_256=stride_bytes_256,
            gen_mode=0,
            single_packet=single_packet,
            queue_num=0,
            sbuf_tokens_per_rank=0,
            sbuf_free_dim_per_rank=0,
            sbuf_free_dim_pad_per_rank=0,
            sbuf_byte_offset=0,
        )
    )


def _b(ap2, reps):
    """broadcast each element of an AP over `reps` trailing copies."""
    return bass.AP(tensor=ap2.tensor, offset=ap2.offset, ap=[*ap2.ap, [0, reps]])


def _bcast_col(ap1, n):
    """[P, 1] AP -> [P, n] zero-stride broadcast."""
    return bass.AP(tensor=ap1.tensor, offset=ap1.offset, ap=[ap1.ap[0], [0, n]])


def build(meta):
    geo: Geo = meta["geo"]
    S_T, COLS, T, KC = meta["S_T"], meta["COLS"], meta["T"], meta["KC"]
    colmeta, ginstr = meta["colmeta"], meta["ginstr"]
    CB, KB = meta["CB"], meta["KB"]
    nsh, ngrp, nwin = geo.nsh, geo.ngrp, geo.nwin
    f32 = mybir.dt.float32
    AX = mybir.AxisListType.X
    AF = mybir.ActivationFunctionType

    nc = bacc.Bacc("TRN2", target_bir_lowering=False, debug=False, num_devices=CORES)

    xT = nc.dram_tensor("xT", [F_IN, nsh], f32, kind="ExternalInput")
    gidx = nc.dram_tensor("gidx", [128, T // 16], mybir.dt.int16, kind="ExternalInput")
    w4g = nc.dram_tensor("w4g", [128, KC * 4], f32, kind="ExternalInput")
    dloc = nc.dram_tensor("dloc", [128, COLS], f32, kind="ExternalInput")
    iota512 = nc.dram_tensor("iota512", [128, 1024], f32, kind="ExternalInput")
    ident = nc.dram_tensor("ident", [128, 128], f32, kind="ExternalInput")
    W1t = nc.dram_tensor("W1t", [F_IN, F_HID], f32, kind="ExternalInput")
    b1t = nc.dram_tensor("b1t", [128, F_HID], f32, kind="ExternalInput")
    W2t = nc.dram_tensor("W2t", [128, F_OUT], f32, kind="ExternalInput")
    b2t = nc.dram_tensor("b2t", [128, F_OUT], f32, kind="ExternalInput")
    onest = nc.dram_tensor("onest", [1, 128], f32, kind="ExternalInput")
    out_t = nc.dram_tensor("out", [nsh, F_OUT], f32, kind="ExternalOutput")

    with tile.TileContext(nc) as tc:
        with (
            tc.tile_pool(name="const", bufs=1) as cpool,
            tc.tile_pool(name="dram", bufs=1, space="DRAM") as dram,
            tc.tile_pool(name="gxp", bufs=8) as gxp,
            tc.tile_pool(name="work", bufs=4) as wp,
            tc.tile_pool(name="scol", bufs=8) as sp,
            tc.tile_pool(name="agg", bufs=1) as apool,
        ):
            iota_t = cpool.tile([128, 1024], f32)
            nc.sync.dma_start(out=iota_t[:], in_=iota512[:, :])
            dloc_t = cpool.tile([128, COLS], f32)
            nc.sync.dma_start(out=dloc_t[:], in_=dloc[:, :])
            W1s = cpool.tile([F_IN, F_HID], f32)
            nc.sync.dma_start(out=W1s[:], in_=W1t[:, :])
            b1s = cpool.tile([128, F_HID], f32)
            nc.sync.dma_start(out=b1s[:], in_=b1t[:, :])
            W2s = cpool.tile([128, F_OUT], f32)
            nc.sync.dma_start(out=W2s[:], in_=W2t[:, :])
            b2s = cpool.tile([128, F_OUT], f32)
            nc.sync.dma_start(out=b2s[:], in_=b2t[:, :])
            ones_s = cpool.tile([1, 128], f32)
            nc.sync.dma_start(out=ones_s[:], in_=onest[:, :])
            id_s = cpool.tile([128, 128], f32)
            nc.sync.dma_start(out=id_s[:], in_=ident[:, :])

            shard1 = dram.tile([nsh // 4, 128], f32)
            shard2 = dram.tile([nsh // 4, 128], f32)
            table1 = dram.tile([ngrp, 128], f32, addr_space="Shared")
            table2 = dram.tile([ngrp, 128], f32, addr_space="Shared")

            # ---- h = x @ W1 on own shard -> shard1 (dense [nsh, 32]) ----
            with (
                tc.tile_pool(name="xt", bufs=2) as xp,
                tc.tile_pool(name="ph", bufs=4, space="PSUM") as ph,
            ):
                ntiles = nsh // 128
                quarter = ((ntiles + 3) // 4) * 128
                for hh in range(4):
                    hi = min(quarter * (hh + 1), nsh)
                    lo = quarter * hh
                    if hi <= lo:
                        continue
                    xTs = xp.tile([F_IN, quarter], f32, tag="xts")
                    nc.sync.dma_start(out=xTs[:, : hi - lo], in_=xT[:, lo:hi])
                    tt = 0
                    while tt * 128 < hi - lo:
                        # batch 4 matmul outputs into one 16KB-contig write
                        nb = min(4, (hi - lo) // 128 - tt)
                        hs = wp.tile([128, 4 * F_HID], f32, tag="hsb")
                        for q in range(nb):
                            hp = ph.tile([128, F_HID], f32, tag="hps")
                            nc.tensor.matmul(
                                out=hp[:], lhsT=xTs[:, (tt + q) * 128 : (tt + q + 1) * 128],
                                rhs=W1s[:], start=True, stop=True,
                            )
                            nc.vector.tensor_copy(
                                out=hs[:, q * F_HID : (q + 1) * F_HID], in_=hp[:]
                            )
                        t = (lo // 128) + tt
                        dst = bass.AP(
                            tensor=shard1.tensor, offset=t * 128 * 32,
                            ap=[[32, 128], [128 * 32, nb], [1, 32]],
                        )
                        nc.sync.dma_start(
                            out=dst,
                            in_=hs[:, : nb * F_HID].rearrange("p (b f) -> p b f", f=32),
                        )
                        tt += nb

            nc.gpsimd.collective_compute(
                "AllGather", mybir.AluOpType.bypass,
                ins=[shard1.opt()], outs=[table1.opt()],
                replica_groups=[list(range(CORES))],
            )

            def layer(table, agg_tile, init_b, ps2, transposed=False):
                if init_b is not None:
                    bsrc = bass.AP(
                        tensor=init_b.tensor, offset=init_b[:].offset,
                        ap=[init_b[:].ap[0], [0, nwin], [1, 32]],
                    )
                    nc.vector.tensor_copy(
                        out=agg_tile[:].rearrange("p (v f) -> p v f", f=32), in_=bsrc
                    )
                else:
                    nc.vector.memset(agg_tile[:], 0.0)
                for kc0, gk, tier in ginstr:
                    nid = gk * 128          # idx per instruction
                    gx = gxp.tile([128, gk * 8], mybir.dt.int16, tag="gx")
                    nc.sync.dma_start(out=gx[:], in_=gidx[:, kc0 * 8 : (kc0 + gk) * 8])
                    w4 = gxp.tile([128, gk * 4], f32, tag="w4")
                    nc.sync.dma_start(out=w4[:], in_=w4g[:, kc0 * 4 : (kc0 + gk) * 4])
                    msgs = wp.tile([128, gk * 128], f32, tag="msgs")
                    emit_dma_gather(
                        nc.gpsimd,
                        out_ap=msgs[:].rearrange("p (k f) -> p k f", f=128),
                        in_ap=bass.AP(
                            tensor=table.tensor, offset=0,
                            ap=[[128, ngrp], [1, 128]],
                        ),
                        idxs_ap=gx[:],
                        num_idxs=nid,
                        elem_size=128,
                        elem_step=128,
                    )
                    # mask-weight multiply: [p, k*4, 32] *= w4 bcast over 32
                    nc.vector.tensor_tensor(
                        out=msgs[:].rearrange("p (q f) -> p q f", f=32),
                        in0=msgs[:].rearrange("p (q f) -> p q f", f=32),
                        in1=_b(w4[:], 32), op=mybir.AluOpType.mult,
                    )
                    def pairsum(dst_tile, src_tile, nblk):
                        # dst[i] = src[2i] + src[2i+1] over nblk 32-f blocks
                        nc.vector.tensor_tensor(
                            out=dst_tile[:].rearrange("p (q f) -> p q f", f=32),
                            in0=bass.AP(tensor=src_tile.tensor, offset=src_tile[:].offset,
                                        ap=[src_tile[:].ap[0], [64, nblk], [1, 32]]),
                            in1=bass.AP(tensor=src_tile.tensor, offset=src_tile[:].offset + 32,
                                        ap=[src_tile[:].ap[0], [64, nblk], [1, 32]]),
                            op=mybir.AluOpType.add,
                        )
                    s2 = wp.tile([128, gk * 64], f32, tag="s2")
                    pairsum(s2, msgs, gk * 2)
                    s1 = wp.tile([128, gk * 32], f32, tag="s1")
                    pairsum(s1, s2, gk)
                    # slot-sum: log2(tier) further pairsums
                    out1 = s1
                    nblk = gk // 2
                    d = 0
                    while (1 << d) < tier:
                        nxt = wp.tile([128, nblk * 32], f32, tag=f"tr{tier}_{d}")
                        pairsum(nxt, out1, nblk)
                        out1 = nxt
                        nblk //= 2
                        d += 1
                    # stage 2: per slot-column one-hot matmul into agg windows
                    for sc in range(gk // tier):
                        col = CB[tier] + (kc0 - KB[tier]) // tier + sc
                        cm = colmeta[col]
                        if cm is None:
                            continue
                        a_col, wins = cm
                        for wv in wins:
                            oh = sp.tile([128, 128], f32, tag="oh")
                            nc.vector.tensor_tensor(
                                out=oh[:],
                                in0=iota_t[:, (wv - a_col) * 128 : (wv - a_col + 1) * 128],
                                in1=_bcast_col(dloc_t[:, col : col + 1], 128),
                                op=mybir.AluOpType.is_equal,
                            )
                            if transposed:
                                pw = ps2.tile([32, 128], f32, tag="pwT")
                                nc.tensor.matmul(
                                    out=pw[:], lhsT=out1[:, sc * 32 : (sc + 1) * 32],
                                    rhs=oh[:], start=True, stop=True,
                                )
                                nc.vector.tensor_tensor(
                                    out=agg_tile[:, wv * 128 : (wv + 1) * 128],
                                    in0=agg_tile[:, wv * 128 : (wv + 1) * 128],
                                    in1=pw[:], op=mybir.AluOpType.add,
                                )
                            else:
                                pw = ps2.tile([128, 32], f32, tag="pw")
                                nc.tensor.matmul(
                                    out=pw[:], lhsT=oh[:],
                                    rhs=out1[:, sc * 32 : (sc + 1) * 32],
                                    start=True, stop=True,
                                )
                                nc.vector.tensor_tensor(
                                    out=agg_tile[:, wv * 32 : (wv + 1) * 32],
                                    in0=agg_tile[:, wv * 32 : (wv + 1) * 32],
                                    in1=pw[:], op=mybir.AluOpType.add,
                                )

            # ---- layer 1 ----
            agg1 = apool.tile([128, nwin * 32], f32, tag="agg1")
            with tc.tile_pool(name="ps2a", bufs=6, space="PSUM") as ps2:
                layer(table1, agg1, b1s, ps2)
            # relu -> shard2 (dense [nsh, 32]) -> AllGather -> table2
            h2cm = tc.tile_pool(name="h2p", bufs=1)
            h2pool = h2cm.__enter__()
            h2 = h2pool.tile([128, nwin * 32], f32, tag="h2")
            nc.scalar.activation(
                out=h2[:].rearrange("p (v f) -> p v f", f=32),
                in_=agg1[:].rearrange("p (v f) -> p v f", f=32), func=AF.Relu,
            )
            dst2 = bass.AP(tensor=shard2.tensor, offset=0,
                           ap=[[32, 128], [128 * 32, nwin], [1, 32]])
            nc.sync.dma_start(out=dst2, in_=h2[:].rearrange("p (v f) -> p v f", f=32))
            h2cm.__exit__(None, None, None)
            nc.gpsimd.collective_compute(
                "AllGather", mybir.AluOpType.bypass,
                ins=[shard2.opt()], outs=[table2.opt()],
                replica_groups=[list(range(CORES))],
            )

            # ---- layer 2 (transposed aggregate: [32, nwin*128]) ----
            agg2 = apool.tile([32, nwin * 128], f32, tag="agg2T")
            with tc.tile_pool(name="ps2b", bufs=6, space="PSUM") as ps2:
                layer(table2, agg2, None, ps2, transposed=True)

            # ---- out = log_softmax(agg2 @ W2 + b2) ----
            zall = apool.tile([128, nwin * F_OUT], f32, tag="zall")
            sall = apool.tile([128, nwin], f32, tag="sall")
            pf_cm = tc.tile_pool(name="pf", bufs=3, space="PSUM")
            pf = pf_cm.__enter__()
            for v in range(nwin):
                zp = pf.tile([128, F_OUT], f32, tag="zp")
                nc.tensor.matmul(out=zp[:], lhsT=agg2[:, v * 128 : (v + 1) * 128],
                                 rhs=W2s[:32, :], start=True, stop=True)
                zb = sp.tile([128, F_OUT], f32, tag="zb")
                nc.vector.tensor_tensor(
                    out=zb[:], in0=zp[:], in1=b2s[:], op=mybir.AluOpType.add,
                )
                negm = sp.tile([128, 1], f32, tag="negm")
                nc.vector.reduce_max(out=negm[:], in_=zb[:], axis=AX, negate=True)
                nc.vector.tensor_tensor(
                    out=zall[:, v * F_OUT : (v + 1) * F_OUT],
                    in0=zb[:], in1=_bcast_col(negm[:], F_OUT),
                    op=mybir.AluOpType.add,
                )
                etmp = sp.tile([128, F_OUT], f32, tag="etmp")
                nc.scalar.activation(
                    out=etmp[:], in_=zall[:, v * F_OUT : (v + 1) * F_OUT],
                    func=AF.Exp, accum_out=sall[:, v : v + 1],
                )
            lns = apool.tile([128, nwin], f32, tag="lns")
            nc.scalar.activation(out=lns[:], in_=sall[:], func=AF.Ln)
            nc.vector.tensor_tensor(
                out=zall[:].rearrange("p (v f) -> p v f", f=F_OUT),
                in0=zall[:].rearrange("p (v f) -> p v f", f=F_OUT),
                in1=_b(lns[:], F_OUT),
                op=mybir.AluOpType.subtract,
            )
            outdst = bass.AP(
                tensor=out_t, offset=0,
                ap=[[F_OUT, 128], [128 * F_OUT, nwin], [1, F_OUT]],
            )
            nc.sync.dma_start(out=outdst, in_=zall[:].rearrange("p (v f) -> p v f", f=F_OUT))
            pf_cm.__exit__(None, None, None)

    nc.compile()
    return nc


def make_inmaps(meta, inmaps_edges, x, W1, b1, W2, b2):
    geo: Geo = meta["geo"]
    nsh = geo.nsh
    n = geo.n_nodes
    xT_full = np.zeros((F_IN, geo.ntab), np.float32)
    xT_full[:, :n] = np.asarray(x, np.float32).T
    iota = np.tile(np.arange(1024, dtype=np.float32)[None, :], (128, 1))
    ident = np.eye(128, dtype=np.float32)
    b1b = np.tile(np.asarray(b1, np.float32)[None, :], (128, 1))
    consts = dict(
        iota512=iota, ident=ident,
        W1t=np.asarray(W1, np.float32), b1t=b1b,
        W2t=np.tile(np.asarray(W2, np.float32), (4, 1)),
        b2t=np.tile(np.asarray(b2, np.float32)[None, :], (128, 1)),
        onest=np.ones((1, 128), np.float32),
    )
    maps = []
    for c in range(CORES):
        m = dict(inmaps_edges[c])
        m.update(consts)
        m["xT"] = np.ascontiguousarray(xT_full[:, c * nsh : (c + 1) * nsh])
        maps.append(m)
    return maps


_CACHE = {}


def run(x, edge_index, edge_weight, W1, b1, W2, b2, geo=FULL, trace=False):
    meta, inmaps_edges = pack(edge_index, edge_weight, geo)
    key = ("geo%d" % geo.n_nodes, meta["S_T"])
    if key in _CACHE:
        nc = _CACHE[key]
    else:
        nc = build(meta)
        _CACHE[key] = nc
    maps = make_inmaps(meta, inmaps_edges, x, W1, b1, W2, b2)
    res = run_bass_kernel_spmd(nc, maps, core_ids=list(range(CORES)), trace=trace)
    n = geo.n_nodes
    out = np.empty((n, F_OUT), np.float32)
    for c in range(CORES):
        lo = c * geo.nsh
        hi = min(lo + geo.nsh, n)
        if hi > lo:
            out[lo:hi] = res.results[c]["out"][: hi - lo]
    return out, res


def kernel(x, edge_index, edge_weight, W1, b1, W2, b2):
    out, _ = run(
        np.asarray(x), np.asarray(edge_index), np.asarray(edge_weight),
        np.asarray(W1), np.asarray(b1), np.asarray(W2), np.asarray(b2),
    )
    return out
